# revision 1
# baseline (speedup 1.0000x reference)
"""Trainium2 Bass kernel for nn_ALMSLayer (gnn_message_passing), 8 NeuronCores.

Algorithm (per core c, rows R_c = [c*1024, (c+1)*1024) of B=8192):
  x       = f / ||f||                      (rows normalized)
  sim     = x_c @ x^T                      (bf16 matmul, [1024, 8192])
  topk    : per row, 33rd/34th-largest via chunked max8 candidates ->
            threshold t; M' = (sim >= t) in {0,1} (includes self edge)
  A       = (M' + M'^T)/32 - I/16          (-I/16 corrects the self edges)
  diff1   = A @ f ; geodesic = A @ diff1   (dense bf16 matmuls; M'^T side
            summed across cores with ReduceScatter, M' side local, full
            tensors rebuilt with a bf16 AllGather)
  z       = x + 0.1 * geodesic/||geodesic||
  out     = softmax((x_c @ z^T)/0.1) @ f   (flash-attention style)

Host side only shards/replicates/concats; all arithmetic on device.
"""
import sys

sys.path.insert(0, "/opt/trn_rl_repo")

import numpy as np

import concourse.bass as bass
import concourse.tile as tile
from concourse import bacc, mybir
from concourse.bass_utils import run_bass_kernel_spmd

F32 = mybir.dt.float32
BF16 = mybir.dt.bfloat16
AF = mybir.ActivationFunctionType
ALU = mybir.AluOpType

B = 8192          # nodes
D = 128           # feature dim
P = 128           # partitions
NCORES = 8
BC = B // NCORES  # rows per core (1024)
NS = B // P       # 64 j-slices of 128 rows
NQ = BC // P      # 8 q-tiles per core
RG = [list(range(NCORES))]
NCACHE = 0       # transposed M slices cached in SBUF across both diffusions

# threshold shift: t'' = t*(1 - 2^-10) so elements equal to the 33rd value
# land strictly above the threshold (bf16 value gaps are either 0 or
# >= ~2^-9 relative, so the shift never misclassifies rank 34).
SHIFT = 1.0 - 2.0 ** -10


def _r(ap):
    return ap.rearrange("p s d -> p (s d)")


def _nat(dram_ap):
    """DRAM [S*P, D] viewed as SBUF-natural [P, S, D] (row j = s*128+p)."""
    return dram_ap.rearrange("(s p) d -> p s d", p=P)


def build():
    nc = bacc.Bacc(None, target_bir_lowering=False, debug=False)

    feat = nc.declare_dram_parameter("feat", [B, D], F32, isOutput=False)
    featc = nc.declare_dram_parameter("featc", [BC, D], F32, isOutput=False)
    out_ext = nc.declare_dram_parameter("out", [BC, D], F32, isOutput=True)

    with tile.TileContext(nc) as tc:
        with (
            tc.tile_pool(name="dram", bufs=1, space="DRAM") as dr,
            tc.tile_pool(name="pers", bufs=1) as pers,
        ):
            # ---------------- DRAM scratch ----------------
            M_dram = dr.tile([BC, B], BF16)
            xb_dram = dr.tile([B, D], BF16)
            xcb_dram = dr.tile([BC, D], BF16)
            s_bounce = [dr.tile([B, D], BF16, name=f"sbounce{i}") for i in range(2)]
            rs_out = [dr.tile([BC, D], BF16, name=f"rsout{i}") for i in range(2)]
            ag_in = [dr.tile([BC, D], BF16, name=f"agin{i}") for i in range(3)]
            ag_out = [
                dr.tile([B, D], BF16, addr_space="Shared", name=f"agout{i}")
                for i in range(3)
            ]

            # ---------------- persistent SBUF ----------------
            identf = pers.tile([P, P], F32)
            ones_col = pers.tile([P, 1], BF16)
            fb32 = pers.tile([P, NS, D], BF16)       # f/32 (gather rhs + phase-8 V)
            fbc32 = pers.tile([P, NQ, D], BF16)      # f_c/32 (scatter-1 stationary)
            xc_nat = pers.tile([P, NQ, D], F32)      # x_c fp32 (z build)
            xcT = pers.tile([P, BC], BF16)           # x_c^T (sim + phase-8)

            nc.vector.memset(identf[:], 1.0)
            nc.gpsimd.affine_select(
                identf[:], identf[:], pattern=[[1, P]], compare_op=ALU.is_equal,
                fill=0.0, base=0, channel_multiplier=-1,
            )
            nc.vector.memset(ones_col[:], 1.0)

            # ================ phase 0: load, normalize, layouts ================
            with tc.tile_pool(name="p0", bufs=1) as p0:
                f_sb = p0.tile([P, NS, D], F32)
                nc.scalar.dma_start(f_sb[:], _nat(feat[:]))
                fc_sb = p0.tile([P, NQ, D], F32)
                nc.scalar.dma_start(fc_sb[:], _nat(featc[:]))

                # row norms via DVE squares with accumulate
                n2 = p0.tile([P, NS], F32)
                for s in range(NS):
                    sq = p0.tile([P, D], F32, tag="sq", bufs=2)
                    nc.vector.scalar_tensor_tensor(
                        sq[:], f_sb[:, s, :], 1.0, f_sb[:, s, :],
                        ALU.mult, ALU.mult, accum_out=n2[:, s:s + 1],
                    )
                n2c = p0.tile([P, NQ], F32)
                for q in range(NQ):
                    sq = p0.tile([P, D], F32, tag="sq", bufs=2)
                    nc.vector.scalar_tensor_tensor(
                        sq[:], fc_sb[:, q, :], 1.0, fc_sb[:, q, :],
                        ALU.mult, ALU.mult, accum_out=n2c[:, q:q + 1],
                    )
                nrm = p0.tile([P, NS], F32)
                nc.scalar.activation(nrm[:], n2[:], AF.Sqrt)
                rn = p0.tile([P, NS], F32)
                nc.vector.reciprocal(rn[:], nrm[:])
                nrmc = p0.tile([P, NQ], F32)
                nc.scalar.activation(nrmc[:], n2c[:], AF.Sqrt)
                rnc = p0.tile([P, NQ], F32)
                nc.vector.reciprocal(rnc[:], nrmc[:])

                # x (bf16 full, fp32 core rows), transposed copies via DRAM
                xb_nat = p0.tile([P, NS, D], BF16)
                for s in range(NS):
                    nc.scalar.activation(
                        xb_nat[:, s, :], f_sb[:, s, :], AF.Copy,
                        scale=rn[:, s:s + 1],
                    )
                nc.scalar.dma_start(_nat(xb_dram[:]), xb_nat[:])
                for q in range(NQ):
                    nc.vector.tensor_scalar(
                        xc_nat[:, q, :], fc_sb[:, q, :], rnc[:, q:q + 1], None,
                        ALU.mult,
                    )
                xcb = p0.tile([P, NQ, D], BF16)
                nc.scalar.activation(_r(xcb[:]), _r(xc_nat[:]), AF.Copy)
                nc.scalar.dma_start(_nat(xcb_dram[:]), xcb[:])
                nc.sync.dma_start_transpose(xcT[:], xcb_dram[:])

                nc.scalar.activation(_r(fb32[:]), _r(f_sb[:]), AF.Copy, scale=1 / 32)
                nc.scalar.activation(
                    _r(fbc32[:]), _r(fc_sb[:]), AF.Copy, scale=1 / 32
                )

            # ================ phase 2+3: sim, topk threshold, M' ================
            with (
                tc.tile_pool(name="p23", bufs=1) as p23,
                tc.tile_pool(name="ps23", bufs=1, space="PSUM") as psp,
            ):
                xbT = p23.tile([P, B], BF16)
                nc.sync.dma_start_transpose(xbT[:], xb_dram[:])

                for qt in range(NQ):
                    sim_sb = p23.tile([P, 16, 512], BF16, tag="sim", bufs=3)
                    for chp in range(8):
                        pssim = psp.tile([P, 2, 512], F32, tag="pssim", bufs=3)
                        for u in range(2):
                            ch = chp * 2 + u
                            nc.tensor.matmul(
                                pssim[:, u, :],
                                xcT[:, qt * P:(qt + 1) * P],
                                xbT[:, ch * 512:(ch + 1) * 512],
                                start=True, stop=True,
                            )
                        nc.scalar.activation(
                            sim_sb[:].rearrange("p c f -> p (c f)")
                            [:, chp * 1024:(chp + 1) * 1024],
                            pssim[:].rearrange("p c f -> p (c f)"),
                            AF.Copy,
                        )
                    simf = sim_sb[:].rearrange("p c f -> p (c f)")

                    cand = p23.tile([P, 8, 8], BF16, tag="cand", bufs=2)
                    for c in range(8):
                        nc.vector.max(
                            cand[:, c, :], simf[:, c * 1024:(c + 1) * 1024]
                        )
                    candf = cand[:].rearrange("p c f -> p (c f)")
                    m8 = None
                    for rnd in range(5):
                        m8 = p23.tile([P, 8], BF16, tag="m8", bufs=6)
                        nc.vector.max(m8[:], candf)
                        if rnd < 4:
                            nc.vector.match_replace(candf, m8[:], candf, -1e30)
                    th = p23.tile([P, 1], F32, tag="th", bufs=2)
                    nc.vector.tensor_tensor(th[:], m8[:, 0:1], m8[:, 1:2], ALU.add)
                    Mt = p23.tile([P, NS, D], BF16, tag="Mt", bufs=2)
                    if qt % 2 == 0:
                        # ACT route: sigmoid step with bias = -1e9 * t''
                        tneg = p23.tile([P, 1], F32, tag="tneg", bufs=2)
                        nc.vector.tensor_scalar(
                            tneg[:], th[:], -0.5e9 * SHIFT, None, ALU.mult
                        )
                        nc.scalar.activation(
                            _r(Mt[:]), simf, AF.Sigmoid, bias=tneg[:], scale=1e9
                        )
                    else:
                        # DVE route: exact compare sim >= t''
                        tpos = p23.tile([P, 1], F32, tag="tneg", bufs=2)
                        nc.vector.tensor_scalar(
                            tpos[:], th[:], 0.5 * SHIFT, None, ALU.mult
                        )
                        nc.vector.tensor_scalar(
                            _r(Mt[:]), simf, tpos[:], None, ALU.is_ge
                        )
                    nc.scalar.dma_start(M_dram[qt * P:(qt + 1) * P, :], _r(Mt[:]))
                    del simf

            # ================ diffusion (x2) ================
            def diffusion(i, dpool, psp, rhsc_t, den_t, csrc, cscale,
                          gscale=1.0, do_ag=True):
                """one step: returns dc = (A @ src)_rows-of-core (fp32) and the
                AllGathered bf16 full result in ag_out[i].

                rhsc_t [P,NQ,D] bf16: src_c/32 (scatter stationary)
                den_t  [P,NS,D] bf16: src/32   (gather stationary)
                csrc/cscale: merge-time correction, adds cscale*csrc (-src_c/16)
                """
                # ---- scatter: St[d, j] = sum_i (src_c/32)[i, d] * M'[i, j]
                S_sb = dpool.tile([P, NS, D], BF16, tag="Ssb")
                for half in range(2):
                    psSt = [
                        psp.tile([P, 512], F32, tag="acc", bufs=8,
                                 name=f"psSt{i}_{half}_{js}")
                        for js in range(8)
                    ]
                    for qq in range(NQ // 2):
                        Mq = dpool.tile([P, 2, 4096], BF16, tag="Mq", bufs=2)
                        nc.scalar.dma_start(
                            Mq[:].rearrange("p a b -> p (a b)"),
                            M_dram[qq * 2 * P:(qq + 1) * 2 * P,
                                   half * 4096:(half + 1) * 4096].rearrange(
                                       "(a p) j -> p a j", p=P),
                        )
                        for a in range(2):
                            q = qq * 2 + a
                            for js in range(8):
                                nc.tensor.matmul(
                                    psSt[js][:], rhsc_t[:, q, :],
                                    Mq[:, a, js * 512:(js + 1) * 512],
                                    start=(q == 0), stop=(q == NQ - 1),
                                )
                    StT = dpool.tile([P, 8, 512], F32, tag="StT", bufs=1)
                    for js in range(8):
                        nc.vector.tensor_copy(StT[:, js, :], psSt[js][:])
                    StTf = StT[:].rearrange("p a b -> p (a b)")
                    for b in range(32):
                        jg = half * 32 + b
                        psb = psp.tile([P, P], F32, tag="acc", bufs=8,
                                       name=f"ptrS{i}_{jg}")
                        nc.tensor.transpose(
                            psb[:], StTf[:, b * P:(b + 1) * P], identf[:]
                        )
                        if b % 2 == 0:
                            nc.scalar.activation(S_sb[:, jg, :], psb[:], AF.Copy)
                        else:
                            nc.vector.tensor_copy(S_sb[:, jg, :], psb[:])
                nc.scalar.dma_start(_nat(s_bounce[i][:]), S_sb[:])
                nc.gpsimd.collective_compute(
                    "ReduceScatter", ALU.add, replica_groups=RG,
                    ins=[s_bounce[i][:].opt()], outs=[rs_out[i][:].opt()],
                )

                # ---- gather: Gt[d, i] = sum_j (src/32)[j, d] * M'[i, j]
                psGt = [
                    psp.tile([P, 512], F32, tag="acc", bufs=8,
                             name=f"psGt{i}_{h}")
                    for h in range(2)
                ]
                for js in range(NS):
                    if js < NCACHE:
                        MT = mt_cache[js]
                        if i == 0:
                            nc.sync.dma_start_transpose(
                                MT[:], M_dram[:, js * P:(js + 1) * P]
                            )
                    else:
                        MT = dpool.tile([P, BC], BF16, tag="MT", bufs=12)
                        nc.sync.dma_start_transpose(
                            MT[:], M_dram[:, js * P:(js + 1) * P]
                        )
                    for h in range(2):
                        nc.tensor.matmul(
                            psGt[h][:], den_t[:, js, :],
                            MT[:, h * 512:(h + 1) * 512],
                            start=(js == 0), stop=(js == NS - 1),
                        )
                GT = dpool.tile([P, 2, 512], F32, tag="GT", bufs=1)
                for h in range(2):
                    nc.scalar.activation(GT[:, h, :], psGt[h][:], AF.Copy)
                GTf = GT[:].rearrange("p a b -> p (a b)")

                # rs + correction, then dc = G^T-transposed + that
                rs_sb = dpool.tile([P, NQ, D], BF16, tag="rssb", bufs=2)
                nc.scalar.dma_start(rs_sb[:], _nat(rs_out[i][:]))
                rsm = dpool.tile([P, NQ, D], F32, tag="rsm", bufs=2)
                for q in range(NQ):
                    nc.vector.scalar_tensor_tensor(
                        rsm[:, q, :], csrc[:, q, :], cscale, rs_sb[:, q, :],
                        ALU.mult, ALU.add,
                    )
                dc = dpool.tile([P, NQ, D], F32, tag=f"dc{i}")
                dcb = dpool.tile([P, NQ, D], BF16, tag="dcb", bufs=2)
                for q in range(NQ):
                    psb = psp.tile([P, P], F32, tag="acc", bufs=8,
                                   name=f"ptrG{i}_{q}")
                    nc.tensor.transpose(
                        psb[:], GTf[:, q * P:(q + 1) * P], identf[:]
                    )
                    nc.vector.scalar_tensor_tensor(
                        dc[:, q, :], psb[:], gscale, rsm[:, q, :],
                        ALU.mult, ALU.add,
                    )
                if do_ag:
                    nc.scalar.activation(_r(dcb[:]), _r(dc[:]), AF.Copy)
                    nc.scalar.dma_start(_nat(ag_in[i][:]), dcb[:])
                    nc.gpsimd.collective_compute(
                        "AllGather", ALU.bypass, replica_groups=RG,
                        ins=[ag_in[i][:].opt()], outs=[ag_out[i][:].opt()],
                    )
                return dc

            with (
                tc.tile_pool(name="dif", bufs=1) as dpool,
                tc.tile_pool(name="psdif", bufs=1, space="PSUM") as psp,
            ):
                mt_cache = [
                    dpool.tile([P, BC], BF16, tag=f"mtc{j}", name=f"mtc{j}")
                    for j in range(NCACHE)
                ]
                dc1 = diffusion(0, dpool, psp, fbc32, fb32, fbc32, -2.0)
                # operands for pass 2 (diff1 arrives bf16 via AllGather);
                # den2 stays unscaled, the gather merge divides by 32.
                den2 = dpool.tile([P, NS, D], BF16, tag="den2")
                nc.scalar.dma_start(den2[:], _nat(ag_out[0][:]))
                rhsc2 = dpool.tile([P, NQ, D], BF16, tag="rhsc2")
                nc.scalar.activation(_r(rhsc2[:]), _r(dc1[:]), AF.Copy, scale=1 / 32)

                dc2 = diffusion(1, dpool, psp, rhsc2, den2, dc1, -1.0 / 16.0,
                                gscale=1.0 / 32.0, do_ag=False)

                # ---- phase 7: z_c = x_c + 0.1 * geo_c/||geo_c||; AllGather z
                n2g = dpool.tile([P, NQ], F32)
                for q in range(NQ):
                    sq = dpool.tile([P, D], F32, tag="sqg", bufs=2)
                    nc.vector.scalar_tensor_tensor(
                        sq[:], dc2[:, q, :], 1.0, dc2[:, q, :],
                        ALU.mult, ALU.mult, accum_out=n2g[:, q:q + 1],
                    )
                ng = dpool.tile([P, NQ], F32)
                nc.scalar.activation(ng[:], n2g[:], AF.Sqrt)
                rgn = dpool.tile([P, NQ], F32)
                nc.vector.reciprocal(rgn[:], ng[:])
                rg01 = dpool.tile([P, NQ], F32)
                nc.vector.tensor_scalar(rg01[:], rgn[:], 0.1, None, ALU.mult)
                zbc = dpool.tile([P, NQ, D], BF16)
                for q in range(NQ):
                    nc.vector.scalar_tensor_tensor(
                        zbc[:, q, :], dc2[:, q, :], rg01[:, q:q + 1],
                        xc_nat[:, q, :], ALU.mult, ALU.add,
                    )
                nc.scalar.dma_start(_nat(ag_in[2][:]), zbc[:])
                nc.gpsimd.collective_compute(
                    "AllGather", ALU.bypass, replica_groups=RG,
                    ins=[ag_in[2][:].opt()], outs=[ag_out[2][:].opt()],
                )

            # ================ phase 8: softmax attention ================
            with (
                tc.tile_pool(name="p8", bufs=1) as p8,
                tc.tile_pool(name="ps8", bufs=1, space="PSUM") as psp,
            ):
                zT = p8.tile([P, B], BF16)
                for zs in range(8):
                    nc.sync.dma_start_transpose(
                        zT[:, zs * 1024:(zs + 1) * 1024],
                        ag_out[2][zs * 1024:(zs + 1) * 1024, :],
                    )
                for grp in range(2):
                    psOUT = psp.tile([P, 512], F32, tag="psOUT", bufs=1)
                    psS1 = psp.tile([1, 512], F32, tag="psS1", bufs=1)
                    for jc in range(NS):
                        psT = psp.tile([P, 512], F32, tag="psT", bufs=3)
                        nc.tensor.matmul(
                            psT[:], zT[:, jc * P:(jc + 1) * P],
                            xcT[:, grp * 512:(grp + 1) * 512],
                            start=True, stop=True,
                        )
                        Pt = p8.tile([P, 512], BF16, tag="Pt", bufs=4)
                        nc.scalar.activation(Pt[:], psT[:], AF.Exp, scale=10.0)
                        nc.tensor.matmul(
                            psOUT[:], fb32[:, jc, :], Pt[:],
                            start=(jc == 0), stop=(jc == NS - 1),
                        )
                        nc.tensor.matmul(
                            psS1[:], ones_col[:], Pt[:],
                            start=(jc == 0), stop=(jc == NS - 1),
                        )
                    rS = p8.tile([1, 512], F32, tag="rS", bufs=2)
                    nc.vector.reciprocal(rS[:], psS1[:])
                    rS32 = p8.tile([1, 512], F32, tag="rS32", bufs=2)
                    nc.vector.tensor_scalar(rS32[:], rS[:], 32.0, None, ALU.mult)
                    OUT_sb = p8.tile([P, 512], F32, tag="OUTsb", bufs=2)
                    nc.scalar.activation(OUT_sb[:], psOUT[:], AF.Copy)
                    rnat = p8.tile([P, 4], F32, tag="rnat", bufs=2)
                    for b in range(4):
                        psr = psp.tile([P, 1], F32, tag="psr", bufs=1)
                        nc.tensor.transpose(
                            psr[:], rS32[0:1, b * P:(b + 1) * P], identf[0:1, 0:1]
                        )
                        nc.scalar.activation(rnat[:, b:b + 1], psr[:], AF.Copy)
                    for b in range(4):
                        psB = psp.tile([P, P], F32, tag="psB", bufs=2)
                        nc.tensor.transpose(
                            psB[:], OUT_sb[:, b * P:(b + 1) * P], identf[:]
                        )
                        ob = p8.tile([P, D], F32, tag="ob", bufs=2)
                        nc.vector.tensor_scalar(
                            ob[:], psB[:], rnat[:, b:b + 1], None, ALU.mult
                        )
                        nc.scalar.dma_start(
                            out_ext[grp * 512 + b * P: grp * 512 + (b + 1) * P, :],
                            ob[:],
                        )

    nc.finalize()
    return nc


_NC_CACHE = None


def kernel(features: np.ndarray) -> np.ndarray:
    global _NC_CACHE
    features = np.ascontiguousarray(np.asarray(features, np.float32))
    assert features.shape == (B, D), features.shape
    if _NC_CACHE is None:
        _NC_CACHE = build()
    in_maps = [
        {
            "feat": features,
            "featc": features[c * BC:(c + 1) * BC].copy(),
        }
        for c in range(NCORES)
    ]
    res = run_bass_kernel_spmd(_NC_CACHE, in_maps, core_ids=list(range(NCORES)))
    return np.concatenate(
        [np.asarray(res.results[c]["out"], np.float32) for c in range(NCORES)],
        axis=0,
    )



# revision 2
# speedup vs baseline: 1.1967x; 1.1967x over previous
"""Trainium2 Bass kernel for nn_ALMSLayer (gnn_message_passing), 8 NeuronCores.

Algorithm (per core c, rows R_c = [c*1024, (c+1)*1024) of B=8192):
  x       = f / ||f||                      (rows normalized)
  sim     = x_c @ x^T                      (bf16 matmul, [1024, 8192])
  topk    : per row, 33rd/34th-largest via chunked top-8 candidates ->
            threshold t; M' = (sim >= t) in {0,1} (includes self edge)
  A       = (M' + M'^T)/32 - I/16          (-I/16 corrects the self edges)
  diff1   = A @ f ; geodesic = A @ diff1   (dense bf16 matmuls; M'^T side
            summed across cores with ReduceScatter, M' side local, full
            tensors rebuilt with a bf16 AllGather)
  z       = x + 0.1 * geodesic/||geodesic||
  out     = softmax((x_c @ z^T)/0.1) @ f   (flash-attention style)

Key perf structure vs the naive layout:
  * M' is written to DRAM once (natural layout) and read back only for the
    scatter side.  The gather side needs M'^T; instead of transposed DMA
    reads (2-byte-granularity crawl), each M'^T j-slice is recomputed on
    the fly: sim^T[j, i] = x_j . x_i via one PE matmul from xbT/xcT, with
    the per-i threshold applied either by a rank-1 PSUM accumulate plus
    sigmoid step (even slices) or a broadcast-threshold compare (odd).
  * x^T and z^T full tensors are built by AllGathering locally PE-transposed
    [D, 1024] chunks (natural-layout loads), never by DMA transpose.
  * diff2's scatter is issued before diff1's AllGather completes; loads that
    depend on collectives go on the sync queue so the scalar queue never
    blocks.

Host side only shards/replicates/concats; all arithmetic on device.
"""
import sys

sys.path.insert(0, "/opt/trn_rl_repo")

import numpy as np

import concourse.bass as bass
import concourse.tile as tile
from concourse import bacc, mybir
from concourse.bass_utils import run_bass_kernel_spmd

F32 = mybir.dt.float32
BF16 = mybir.dt.bfloat16
AF = mybir.ActivationFunctionType
ALU = mybir.AluOpType

B = 8192          # nodes
D = 128           # feature dim
P = 128           # partitions
NCORES = 8
BC = B // NCORES  # rows per core (1024)
NS = B // P       # 64 j-slices of 128 rows
NQ = BC // P      # 8 q-tiles per core
RG = [list(range(NCORES))]

# threshold shift: t'' = t*(1 - 2^-10) so elements equal to the 33rd value
# land strictly above the threshold (bf16 value gaps are either 0 or
# >= ~2^-9 relative, so the shift never misclassifies rank 34).
SHIFT = 1.0 - 2.0 ** -10


def _r(ap):
    return ap.rearrange("p s d -> p (s d)")


def _nat(dram_ap):
    """DRAM [S*P, D] viewed as SBUF-natural [P, S, D] (row j = s*128+p)."""
    return dram_ap.rearrange("(s p) d -> p s d", p=P)


def build():
    nc = bacc.Bacc(None, target_bir_lowering=False, debug=False)

    feat = nc.declare_dram_parameter("feat", [B, D], F32, isOutput=False)
    featc = nc.declare_dram_parameter("featc", [BC, D], F32, isOutput=False)
    out_ext = nc.declare_dram_parameter("out", [BC, D], F32, isOutput=True)

    with tile.TileContext(nc) as tc:
        with (
            tc.tile_pool(name="dram", bufs=1, space="DRAM") as dr,
            tc.tile_pool(name="pers", bufs=1) as pers,
        ):
            # ---------------- DRAM scratch ----------------
            M_dram = dr.tile([BC, B], BF16)
            s_bounce = [dr.tile([B, D], BF16, name=f"sbounce{i}") for i in range(2)]
            rs_out = [dr.tile([BC, D], BF16, name=f"rsout{i}") for i in range(2)]
            xct_dram = dr.tile([P, BC], BF16)
            zct_dram = dr.tile([P, BC], BF16)
            dcb_dram = dr.tile([BC, D], BF16)
            xbt_ag = dr.tile([NCORES * P, BC], BF16, addr_space="Shared")
            zt_ag = dr.tile([NCORES * P, BC], BF16, addr_space="Shared")
            den_ag = dr.tile([B, D], BF16, addr_space="Shared")

            # ---------------- persistent SBUF ----------------
            identf = pers.tile([P, P], F32)
            identb = pers.tile([P, P], BF16)
            ones_col = pers.tile([P, 1], BF16)
            ones1b = pers.tile([1, P], BF16)
            fb32 = pers.tile([P, NS, D], BF16)       # f/32 (gather rhs + phase-8 V)
            fbc32 = pers.tile([P, NQ, D], BF16)      # f_c/32 (scatter-1 stationary)
            xc_nat = pers.tile([P, NQ, D], F32)      # x_c fp32 (z build)
            xcT = pers.tile([P, BC], BF16)           # x_c^T [d, i]
            xbT = pers.tile([P, B], BF16)            # x^T [d, j] (AllGathered)
            tcol = pers.tile([P, NQ], F32)           # raw th (v33+v34) per q-tile
            negtb = pers.tile([1, BC], BF16)         # -t'' per i (rank-1 rhs)
            T_bc = pers.tile([P, BC], BF16)          # +t'' broadcast to all parts

            nc.vector.memset(identf[:], 1.0)
            nc.gpsimd.affine_select(
                identf[:], identf[:], pattern=[[1, P]], compare_op=ALU.is_equal,
                fill=0.0, base=0, channel_multiplier=-1,
            )
            nc.vector.memset(identb[:], 1.0)
            nc.gpsimd.affine_select(
                identb[:], identb[:], pattern=[[1, P]], compare_op=ALU.is_equal,
                fill=0.0, base=0, channel_multiplier=-1,
            )
            nc.vector.memset(ones_col[:], 1.0)
            nc.vector.memset(ones1b[:], 1.0)

            # ================ phase 0: load, normalize, layouts ================
            with (
                tc.tile_pool(name="p0", bufs=1) as p0,
                tc.tile_pool(name="ps0", bufs=1, space="PSUM") as ps0,
            ):
                f_sb = p0.tile([P, NS, D], F32)
                nc.scalar.dma_start(f_sb[:], _nat(feat[:]))
                fc_sb = p0.tile([P, NQ, D], F32)
                nc.scalar.dma_start(fc_sb[:], _nat(featc[:]))

                # row norms via DVE squares with accumulate
                n2 = p0.tile([P, NS], F32)
                for s in range(NS):
                    sq = p0.tile([P, D], F32, tag="sq", bufs=2)
                    nc.vector.scalar_tensor_tensor(
                        sq[:], f_sb[:, s, :], 1.0, f_sb[:, s, :],
                        ALU.mult, ALU.mult, accum_out=n2[:, s:s + 1],
                    )
                n2c = p0.tile([P, NQ], F32)
                for q in range(NQ):
                    sq = p0.tile([P, D], F32, tag="sq", bufs=2)
                    nc.vector.scalar_tensor_tensor(
                        sq[:], fc_sb[:, q, :], 1.0, fc_sb[:, q, :],
                        ALU.mult, ALU.mult, accum_out=n2c[:, q:q + 1],
                    )
                nrmc = p0.tile([P, NQ], F32)
                nc.scalar.activation(nrmc[:], n2c[:], AF.Sqrt)
                rnc = p0.tile([P, NQ], F32)
                nc.vector.reciprocal(rnc[:], nrmc[:])
                nrm = p0.tile([P, NS], F32)
                nc.scalar.activation(nrm[:], n2[:], AF.Sqrt)
                rn = p0.tile([P, NS], F32)
                nc.vector.reciprocal(rn[:], nrm[:])

                # x_c fp32 + bf16; xcT via PE transposes; AllGather -> xbT
                for q in range(NQ):
                    nc.vector.tensor_scalar(
                        xc_nat[:, q, :], fc_sb[:, q, :], rnc[:, q:q + 1], None,
                        ALU.mult,
                    )
                xcb = p0.tile([P, NQ, D], BF16)
                nc.scalar.activation(_r(xcb[:]), _r(xc_nat[:]), AF.Copy)
                for q in range(NQ):
                    psq = ps0.tile([P, P], BF16, tag="ptr", bufs=2)
                    nc.tensor.transpose(psq[:], xcb[:, q, :], identb[:])
                    nc.vector.tensor_copy(xcT[:, q * P:(q + 1) * P], psq[:])
                nc.scalar.dma_start(xct_dram[:], xcT[:])
                nc.gpsimd.collective_compute(
                    "AllGather", ALU.bypass, replica_groups=RG,
                    ins=[xct_dram[:].opt()], outs=[xbt_ag[:].opt()],
                )

                nc.scalar.activation(_r(fb32[:]), _r(f_sb[:]), AF.Copy, scale=1 / 32)
                nc.scalar.activation(
                    _r(fbc32[:]), _r(fc_sb[:]), AF.Copy, scale=1 / 32
                )
                for c in range(NCORES):
                    nc.sync.dma_start(
                        xbT[:, c * BC:(c + 1) * BC],
                        xbt_ag[c * P:(c + 1) * P, :],
                    )

            # ================ phase 2+3: sim, topk threshold, M' ================
            with (
                tc.tile_pool(name="p23", bufs=1) as p23,
                tc.tile_pool(name="ps23", bufs=1, space="PSUM") as psp,
            ):
                for qt in range(NQ):
                    sim_sb = p23.tile([P, 16, 512], BF16, tag="sim", bufs=2)
                    for chp in range(8):
                        pssim = psp.tile([P, 2, 512], F32, tag="pssim", bufs=3)
                        for u in range(2):
                            ch = chp * 2 + u
                            nc.tensor.matmul(
                                pssim[:, u, :],
                                xcT[:, qt * P:(qt + 1) * P],
                                xbT[:, ch * 512:(ch + 1) * 512],
                                start=True, stop=True,
                            )
                        nc.scalar.activation(
                            sim_sb[:].rearrange("p c f -> p (c f)")
                            [:, chp * 1024:(chp + 1) * 1024],
                            pssim[:].rearrange("p c f -> p (c f)"),
                            AF.Copy,
                        )
                    simf = sim_sb[:].rearrange("p c f -> p (c f)")

                    cand = p23.tile([P, 8, 8], BF16, tag="cand", bufs=2)
                    for c in range(8):
                        nc.vector.max(
                            cand[:, c, :], simf[:, c * 1024:(c + 1) * 1024]
                        )
                    candf = cand[:].rearrange("p c f -> p (c f)")
                    m8 = None
                    for rnd in range(5):
                        m8 = p23.tile([P, 8], BF16, tag="m8", bufs=6)
                        nc.vector.max(m8[:], candf)
                        if rnd < 4:
                            nc.vector.match_replace(candf, m8[:], candf, -1e30)
                    th = p23.tile([P, 1], F32, tag="th", bufs=2)
                    nc.vector.tensor_tensor(th[:], m8[:, 0:1], m8[:, 1:2], ALU.add)
                    nc.vector.tensor_copy(tcol[:, qt:qt + 1], th[:])
                    Mt = p23.tile([P, NS, D], BF16, tag="Mt", bufs=2)
                    if qt % 2 == 0:
                        # ACT route: sigmoid step with bias = -1e9 * t''
                        tneg = p23.tile([P, 1], F32, tag="tneg", bufs=2)
                        nc.vector.tensor_scalar(
                            tneg[:], th[:], -0.5e9 * SHIFT, None, ALU.mult
                        )
                        nc.scalar.activation(
                            _r(Mt[:]), simf, AF.Sigmoid, bias=tneg[:], scale=1e9
                        )
                    else:
                        # DVE route: exact compare sim >= t''
                        tpos = p23.tile([P, 1], F32, tag="tneg", bufs=2)
                        nc.vector.tensor_scalar(
                            tpos[:], th[:], 0.5 * SHIFT, None, ALU.mult
                        )
                        nc.vector.tensor_scalar(
                            _r(Mt[:]), simf, tpos[:], None, ALU.is_ge
                        )
                    nc.scalar.dma_start(M_dram[qt * P:(qt + 1) * P, :], _r(Mt[:]))
                    del simf

            # ---- threshold row layout: negtb [1, BC], T_bc [P, BC] ----
            with (
                tc.tile_pool(name="throw", bufs=1) as trw,
                tc.tile_pool(name="pst", bufs=1, space="PSUM") as pst,
            ):
                tposb = trw.tile([1, BC], BF16)
                for q in range(NQ):
                    ps1 = pst.tile([1, P], F32, tag="t1", bufs=2)
                    nc.tensor.transpose(ps1[:], tcol[:, q:q + 1], identf[:])
                    nc.vector.tensor_scalar(
                        negtb[0:1, q * P:(q + 1) * P], ps1[:],
                        -0.5 * SHIFT, None, ALU.mult,
                    )
                    nc.vector.tensor_scalar(
                        tposb[0:1, q * P:(q + 1) * P], ps1[:],
                        0.5 * SHIFT, None, ALU.mult,
                    )
                for h in range(2):
                    psb = pst.tile([P, 512], F32, tag="tb", bufs=2)
                    nc.tensor.matmul(
                        psb[:], ones1b[:], tposb[0:1, h * 512:(h + 1) * 512],
                        start=True, stop=True,
                    )
                    nc.scalar.activation(
                        T_bc[:, h * 512:(h + 1) * 512], psb[:], AF.Copy
                    )

            # ================ diffusion (x2) ================
            def diffusion(i, dpool, rhsc_t, den_t, den_src, csrc, cscale,
                          gscale=1.0, do_ag=True):
                """one step: returns dc = (A @ src)_rows-of-core (fp32).

                rhsc_t [P,NQ,D] bf16: src_c/32 (scatter stationary)
                den_t  [P,NS,D] bf16: src/32 (or src; gather stationary)
                den_src: if not None, DRAM ap to load den_t from (after the
                         scatter is issued; sync queue so nothing blocks)
                csrc/cscale: merge-time correction, adds cscale*csrc
                """
                # ---- scatter: St[d, j] = sum_i (src_c/32)[i, d] * M'[i, j]
                S_sb = dpool.tile([P, NS, D], BF16, tag="Ssb")
                with tc.tile_pool(name=f"pscat{i}", bufs=1, space="PSUM") as psc:
                    for half in range(2):
                        psSt = [
                            psc.tile([P, 512], F32, tag="acc", bufs=8,
                                     name=f"psSt{i}_{half}_{js}")
                            for js in range(8)
                        ]
                        for qq in range(NQ // 2):
                            Mq = dpool.tile([P, 2, 4096], BF16, tag="Mq", bufs=2)
                            nc.scalar.dma_start(
                                Mq[:].rearrange("p a b -> p (a b)"),
                                M_dram[qq * 2 * P:(qq + 1) * 2 * P,
                                       half * 4096:(half + 1) * 4096].rearrange(
                                           "(a p) j -> p a j", p=P),
                            )
                            for a in range(2):
                                q = qq * 2 + a
                                for js in range(8):
                                    nc.tensor.matmul(
                                        psSt[js][:], rhsc_t[:, q, :],
                                        Mq[:, a, js * 512:(js + 1) * 512],
                                        start=(q == 0), stop=(q == NQ - 1),
                                    )
                        StT = dpool.tile([P, 8, 512], BF16, tag="StT", bufs=1)
                        for js in range(8):
                            nc.scalar.activation(StT[:, js, :], psSt[js][:], AF.Copy)
                        StTf = StT[:].rearrange("p a b -> p (a b)")
                        for b in range(32):
                            jg = half * 32 + b
                            psb = psc.tile([P, P], BF16, tag="acc", bufs=8,
                                           name=f"ptrS{i}_{jg}")
                            nc.tensor.transpose(
                                psb[:], StTf[:, b * P:(b + 1) * P], identb[:]
                            )
                            if b % 2 == 0:
                                nc.scalar.activation(S_sb[:, jg, :], psb[:], AF.Copy)
                            else:
                                nc.vector.tensor_copy(S_sb[:, jg, :], psb[:])
                nc.scalar.dma_start(_nat(s_bounce[i][:]), S_sb[:])
                nc.gpsimd.collective_compute(
                    "ReduceScatter", ALU.add, replica_groups=RG,
                    ins=[s_bounce[i][:].opt()], outs=[rs_out[i][:].opt()],
                )

                if den_src is not None:
                    nc.sync.dma_start(den_t[:], _nat(den_src[:]))

                # ---- gather: Gt[d, i] = sum_j src[j, d] * M'[i, j]
                # M'^T j-slices are recomputed from sim^T = xbT_js^T @ xcT.
                with tc.tile_pool(name=f"pgat{i}", bufs=1, space="PSUM") as psg:
                    psGt = [
                        psg.tile([P, 512], F32, tag="gacc", bufs=2,
                                 name=f"psGt{i}_{h}")
                        for h in range(2)
                    ]
                    for js in range(NS):
                        MT = dpool.tile([P, BC], BF16, tag="MT", bufs=4)
                        for h in range(2):
                            rg = psg.tile([P, 512], F32, tag="rg", bufs=4)
                            if js % 2 == 0:
                                nc.tensor.matmul(
                                    rg[:], xbT[:, js * P:(js + 1) * P],
                                    xcT[:, h * 512:(h + 1) * 512],
                                    start=True, stop=False,
                                )
                                nc.tensor.matmul(
                                    rg[:], ones1b[:],
                                    negtb[0:1, h * 512:(h + 1) * 512],
                                    start=False, stop=True,
                                )
                                nc.scalar.activation(
                                    MT[:, h * 512:(h + 1) * 512], rg[:],
                                    AF.Sigmoid, scale=1e9,
                                )
                            else:
                                nc.tensor.matmul(
                                    rg[:], xbT[:, js * P:(js + 1) * P],
                                    xcT[:, h * 512:(h + 1) * 512],
                                    start=True, stop=True,
                                )
                                nc.vector.tensor_tensor(
                                    MT[:, h * 512:(h + 1) * 512], rg[:],
                                    T_bc[:, h * 512:(h + 1) * 512], ALU.is_ge,
                                )
                        for h in range(2):
                            nc.tensor.matmul(
                                psGt[h][:], den_t[:, js, :],
                                MT[:, h * 512:(h + 1) * 512],
                                start=(js == 0), stop=(js == NS - 1),
                            )
                    GT = dpool.tile([P, 2, 512], F32, tag="GT", bufs=1)
                    for h in range(2):
                        nc.scalar.activation(GT[:, h, :], psGt[h][:], AF.Copy)
                    GTf = GT[:].rearrange("p a b -> p (a b)")

                    # rs + correction, then dc = G^T-transposed + that
                    rs_sb = dpool.tile([P, NQ, D], BF16, tag="rssb", bufs=2)
                    nc.sync.dma_start(rs_sb[:], _nat(rs_out[i][:]))
                    rsm = dpool.tile([P, NQ, D], F32, tag="rsm", bufs=2)
                    for q in range(NQ):
                        nc.vector.scalar_tensor_tensor(
                            rsm[:, q, :], csrc[:, q, :], cscale, rs_sb[:, q, :],
                            ALU.mult, ALU.add,
                        )
                    dc = dpool.tile([P, NQ, D], F32, tag=f"dc{i}")
                    for q in range(NQ):
                        psb = psg.tile([P, P], F32, tag="rg", bufs=4,
                                       name=f"ptrG{i}_{q}")
                        nc.tensor.transpose(
                            psb[:], GTf[:, q * P:(q + 1) * P], identf[:]
                        )
                        nc.vector.scalar_tensor_tensor(
                            dc[:, q, :], psb[:], gscale, rsm[:, q, :],
                            ALU.mult, ALU.add,
                        )
                if do_ag:
                    dcb = dpool.tile([P, NQ, D], BF16, tag="dcb", bufs=2)
                    nc.scalar.activation(_r(dcb[:]), _r(dc[:]), AF.Copy)
                    nc.scalar.dma_start(_nat(dcb_dram[:]), dcb[:])
                    nc.gpsimd.collective_compute(
                        "AllGather", ALU.bypass, replica_groups=RG,
                        ins=[dcb_dram[:].opt()], outs=[den_ag[:].opt()],
                    )
                return dc

            with tc.tile_pool(name="dif", bufs=1) as dpool:
                dc1 = diffusion(0, dpool, fbc32, fb32, None, fbc32, -2.0)
                # operands for pass 2 (diff1 arrives bf16 via AllGather);
                # den2 stays unscaled, the gather merge divides by 32.
                rhsc2 = dpool.tile([P, NQ, D], BF16, tag="rhsc2")
                nc.scalar.activation(_r(rhsc2[:]), _r(dc1[:]), AF.Copy, scale=1 / 32)
                den2 = dpool.tile([P, NS, D], BF16, tag="den2")

                dc2 = diffusion(1, dpool, rhsc2, den2, den_ag, dc1, -1.0 / 16.0,
                                gscale=1.0 / 32.0, do_ag=False)

                # ---- phase 7: z_c = x_c + 0.1 * geo_c/||geo_c||; AllGather z^T
                n2g = dpool.tile([P, NQ], F32)
                for q in range(NQ):
                    sq = dpool.tile([P, D], F32, tag="sqg", bufs=2)
                    nc.vector.scalar_tensor_tensor(
                        sq[:], dc2[:, q, :], 1.0, dc2[:, q, :],
                        ALU.mult, ALU.mult, accum_out=n2g[:, q:q + 1],
                    )
                ng = dpool.tile([P, NQ], F32)
                nc.scalar.activation(ng[:], n2g[:], AF.Sqrt)
                rgn = dpool.tile([P, NQ], F32)
                nc.vector.reciprocal(rgn[:], ng[:])
                rg01 = dpool.tile([P, NQ], F32)
                nc.vector.tensor_scalar(rg01[:], rgn[:], 0.1, None, ALU.mult)
                zbc = dpool.tile([P, NQ, D], BF16)
                for q in range(NQ):
                    nc.vector.scalar_tensor_tensor(
                        zbc[:, q, :], dc2[:, q, :], rg01[:, q:q + 1],
                        xc_nat[:, q, :], ALU.mult, ALU.add,
                    )
                zcT = dpool.tile([P, BC], BF16, tag="zcT")
                with tc.tile_pool(name="psz", bufs=1, space="PSUM") as psz:
                    for q in range(NQ):
                        psq = psz.tile([P, P], BF16, tag="ptr", bufs=2)
                        nc.tensor.transpose(psq[:], zbc[:, q, :], identb[:])
                        nc.vector.tensor_copy(zcT[:, q * P:(q + 1) * P], psq[:])
                nc.scalar.dma_start(zct_dram[:], zcT[:])
                nc.gpsimd.collective_compute(
                    "AllGather", ALU.bypass, replica_groups=RG,
                    ins=[zct_dram[:].opt()], outs=[zt_ag[:].opt()],
                )

            # ================ phase 8: softmax attention ================
            with (
                tc.tile_pool(name="p8", bufs=1) as p8,
                tc.tile_pool(name="ps8", bufs=1, space="PSUM") as psp,
            ):
                zT = p8.tile([P, B], BF16)
                for c in range(NCORES):
                    nc.sync.dma_start(
                        zT[:, c * BC:(c + 1) * BC],
                        zt_ag[c * P:(c + 1) * P, :],
                    )
                for grp in range(2):
                    psOUT = psp.tile([P, 512], F32, tag="psOUT", bufs=1)
                    psS1 = psp.tile([1, 512], F32, tag="psS1", bufs=1)
                    for jc in range(NS):
                        psT = psp.tile([P, 512], F32, tag="psT", bufs=3)
                        nc.tensor.matmul(
                            psT[:], zT[:, jc * P:(jc + 1) * P],
                            xcT[:, grp * 512:(grp + 1) * 512],
                            start=True, stop=True,
                        )
                        Pt = p8.tile([P, 512], BF16, tag="Pt", bufs=4)
                        nc.scalar.activation(Pt[:], psT[:], AF.Exp, scale=10.0)
                        nc.tensor.matmul(
                            psOUT[:], fb32[:, jc, :], Pt[:],
                            start=(jc == 0), stop=(jc == NS - 1),
                        )
                        nc.tensor.matmul(
                            psS1[:], ones_col[:], Pt[:],
                            start=(jc == 0), stop=(jc == NS - 1),
                        )
                    rS = p8.tile([1, 512], F32, tag="rS", bufs=2)
                    nc.vector.reciprocal(rS[:], psS1[:])
                    rS32 = p8.tile([1, 512], F32, tag="rS32", bufs=2)
                    nc.vector.tensor_scalar(rS32[:], rS[:], 32.0, None, ALU.mult)
                    OUT_sb = p8.tile([P, 512], F32, tag="OUTsb", bufs=2)
                    nc.scalar.activation(OUT_sb[:], psOUT[:], AF.Copy)
                    rnat = p8.tile([P, 4], F32, tag="rnat", bufs=2)
                    for b in range(4):
                        psr = psp.tile([P, 1], F32, tag="psr", bufs=1)
                        nc.tensor.transpose(
                            psr[:], rS32[0:1, b * P:(b + 1) * P], identf[0:1, 0:1]
                        )
                        nc.scalar.activation(rnat[:, b:b + 1], psr[:], AF.Copy)
                    for b in range(4):
                        psB = psp.tile([P, P], F32, tag="psB", bufs=2)
                        nc.tensor.transpose(
                            psB[:], OUT_sb[:, b * P:(b + 1) * P], identf[:]
                        )
                        ob = p8.tile([P, D], F32, tag="ob", bufs=2)
                        nc.vector.tensor_scalar(
                            ob[:], psB[:], rnat[:, b:b + 1], None, ALU.mult
                        )
                        nc.scalar.dma_start(
                            out_ext[grp * 512 + b * P: grp * 512 + (b + 1) * P, :],
                            ob[:],
                        )

    nc.finalize()
    return nc


_NC_CACHE = None


def kernel(features: np.ndarray) -> np.ndarray:
    global _NC_CACHE
    features = np.ascontiguousarray(np.asarray(features, np.float32))
    assert features.shape == (B, D), features.shape
    if _NC_CACHE is None:
        _NC_CACHE = build()
    in_maps = [
        {
            "feat": features,
            "featc": features[c * BC:(c + 1) * BC].copy(),
        }
        for c in range(NCORES)
    ]
    res = run_bass_kernel_spmd(_NC_CACHE, in_maps, core_ids=list(range(NCORES)))
    return np.concatenate(
        [np.asarray(res.results[c]["out"], np.float32) for c in range(NCORES)],
        axis=0,
    )


# revision 10
# speedup vs baseline: 1.2499x; 1.0445x over previous
"""Trainium2 Bass kernel for nn_ALMSLayer (gnn_message_passing), 8 NeuronCores.

Algorithm (per core c, rows R_c = [c*1024, (c+1)*1024) of B=8192):
  x       = f / ||f||                      (rows normalized)
  sim     = x_c @ x^T                      (bf16 matmul, [1024, 8192])
  topk    : per row, 33rd/34th-largest via chunked top-8 candidates ->
            threshold t; M' = (sim >= t) in {0,1} (includes self edge)
  A       = (M' + M'^T)/32 - I/16          (-I/16 corrects the self edges)
  diff1   = A @ f ; geodesic = A @ diff1   (dense bf16 matmuls; M'^T side
            summed across cores with ReduceScatter, M' side local, full
            tensors rebuilt with a bf16 AllGather)
  z       = x + 0.1 * geodesic/||geodesic||
  out     = softmax((x_c @ z^T)/0.1) @ f   (flash-attention style)

Key perf structure vs the naive layout:
  * M' is written to DRAM once (natural layout) and read back only for the
    scatter side.  The gather side needs M'^T; instead of transposed DMA
    reads (2-byte-granularity crawl), each M'^T j-slice is recomputed on
    the fly: sim^T[j, i] = x_j . x_i via one PE matmul from xbT/xcT, with
    the per-i threshold applied either by a rank-1 PSUM accumulate plus
    sigmoid step (even slices) or a broadcast-threshold compare (odd).
  * x^T and z^T full tensors are built by AllGathering locally PE-transposed
    [D, 1024] chunks (natural-layout loads), never by DMA transpose.
  * diff2's scatter is issued before diff1's AllGather completes; loads that
    depend on collectives go on the sync queue so the scalar queue never
    blocks.

Host side only shards/replicates/concats; all arithmetic on device.
"""
import sys

sys.path.insert(0, "/opt/trn_rl_repo")

import numpy as np

import concourse.bass as bass
import concourse.tile as tile
from concourse import bacc, mybir
from concourse.bass_utils import run_bass_kernel_spmd

F32 = mybir.dt.float32
BF16 = mybir.dt.bfloat16
FP8 = mybir.dt.float8e4
AF = mybir.ActivationFunctionType
ALU = mybir.AluOpType

B = 8192          # nodes
D = 128           # feature dim
P = 128           # partitions
NCORES = 8
BC = B // NCORES  # rows per core (1024)
NS = B // P       # 64 j-slices of 128 rows
NQ = BC // P      # 8 q-tiles per core
RG = [list(range(NCORES))]

# threshold shift: t'' = t*(1 - 2^-10) so elements equal to the 33rd value
# land strictly above the threshold (bf16 value gaps are either 0 or
# >= ~2^-9 relative, so the shift never misclassifies rank 34).
SHIFT = 1.0 - 2.0 ** -10


def _r(ap):
    return ap.rearrange("p s d -> p (s d)")


def _nat(dram_ap):
    """DRAM [S*P, D] viewed as SBUF-natural [P, S, D] (row j = s*128+p)."""
    return dram_ap.rearrange("(s p) d -> p s d", p=P)


def build():
    nc = bacc.Bacc(None, target_bir_lowering=False, debug=False)

    feat = nc.declare_dram_parameter("feat", [B, D], F32, isOutput=False)
    featc = nc.declare_dram_parameter("featc", [BC, D], F32, isOutput=False)
    out_ext = nc.declare_dram_parameter("out", [BC, D], F32, isOutput=True)

    with tile.TileContext(nc) as tc:
        with (
            tc.tile_pool(name="dram", bufs=1, space="DRAM") as dr,
            tc.tile_pool(name="pers", bufs=1) as pers,
        ):
            # ---------------- DRAM scratch ----------------
            M_dram = dr.tile([BC, B], BF16)
            s_bounce = [dr.tile([B, D], BF16, name=f"sbounce{i}") for i in range(2)]
            rs_out = [dr.tile([BC, D], BF16, name=f"rsout{i}") for i in range(2)]
            zct_dram = dr.tile([P, BC], BF16)
            dcb_dram = dr.tile([BC, D], BF16)
            zt_ag = dr.tile([NCORES * P, BC], BF16, addr_space="Shared")
            den_ag = dr.tile([B, D], BF16, addr_space="Shared")

            # ---------------- persistent SBUF ----------------
            identf = pers.tile([P, P], F32)
            identb = pers.tile([P, P], BF16)
            ones_col = pers.tile([P, 1], BF16)
            ones1b = pers.tile([1, P], BF16)
            fb32 = pers.tile([P, NS, D], BF16)       # f/32 (gather rhs + phase-8 V)
            fbc32 = pers.tile([P, NQ, D], BF16)      # f_c/32 (scatter-1 stationary)
            xc_nat = pers.tile([P, NQ, D], F32)      # x_c fp32 (z build)
            xcT = pers.tile([P, BC], BF16)           # x_c^T [d, i]
            xbT = pers.tile([P, B], BF16)            # x^T [d, j] (AllGathered)
            tcol = pers.tile([P, NQ], F32)           # raw th (v33+v34) per q-tile
            negtb = pers.tile([1, BC], BF16)         # -t'' per i (rank-1 rhs)
            T_bc = pers.tile([P, BC], BF16)          # +t'' broadcast to all parts
            # transposed-mask cache: M'^T j-slices in fp8 (0/1 exact), built
            # during diffusion-1's gather, reused verbatim by diffusion-2
            mtc = [
                pers.tile([P, BC], FP8, name=f"mtc{js}") for js in range(NS)
            ]

            nc.vector.memset(identf[:], 1.0)
            nc.gpsimd.affine_select(
                identf[:], identf[:], pattern=[[1, P]], compare_op=ALU.is_equal,
                fill=0.0, base=0, channel_multiplier=-1,
            )
            nc.vector.memset(identb[:], 1.0)
            nc.gpsimd.affine_select(
                identb[:], identb[:], pattern=[[1, P]], compare_op=ALU.is_equal,
                fill=0.0, base=0, channel_multiplier=-1,
            )
            nc.vector.memset(ones_col[:], 1.0)
            nc.vector.memset(ones1b[:], 1.0)

            # ================ phase 0: load, normalize, layouts ================
            with (
                tc.tile_pool(name="p0", bufs=1) as p0,
                tc.tile_pool(name="ps0", bufs=1, space="PSUM") as ps0,
            ):
                fc_sb = p0.tile([P, NQ, D], F32)
                nc.scalar.dma_start(fc_sb[:], _nat(featc[:]))
                f_sb = p0.tile([P, NS, D], F32)
                nc.scalar.dma_start(f_sb[:], _nat(feat[:]))

                # row norms via DVE squares with accumulate
                n2c = p0.tile([P, NQ], F32)
                for q in range(NQ):
                    sq = p0.tile([P, D], F32, tag="sq", bufs=2)
                    nc.vector.scalar_tensor_tensor(
                        sq[:], fc_sb[:, q, :], 1.0, fc_sb[:, q, :],
                        ALU.mult, ALU.mult, accum_out=n2c[:, q:q + 1],
                    )
                nrmc = p0.tile([P, NQ], F32)
                nc.scalar.activation(nrmc[:], n2c[:], AF.Sqrt)
                rnc = p0.tile([P, NQ], F32)
                nc.vector.reciprocal(rnc[:], nrmc[:])
                n2 = p0.tile([P, NS], F32)
                for s in range(NS):
                    sq = p0.tile([P, D], F32, tag="sq", bufs=2)
                    nc.vector.scalar_tensor_tensor(
                        sq[:], f_sb[:, s, :], 1.0, f_sb[:, s, :],
                        ALU.mult, ALU.mult, accum_out=n2[:, s:s + 1],
                    )
                nrm = p0.tile([P, NS], F32)
                nc.scalar.activation(nrm[:], n2[:], AF.Sqrt)
                rn = p0.tile([P, NS], F32)
                nc.vector.reciprocal(rn[:], nrm[:])

                # x_c fp32 + bf16; xcT via PE transposes
                for q in range(NQ):
                    nc.vector.tensor_scalar(
                        xc_nat[:, q, :], fc_sb[:, q, :], rnc[:, q:q + 1], None,
                        ALU.mult,
                    )
                xcb = p0.tile([P, NQ, D], BF16)
                nc.scalar.activation(_r(xcb[:]), _r(xc_nat[:]), AF.Copy)
                for q in range(NQ):
                    psq = ps0.tile([P, P], BF16, tag="ptr", bufs=2)
                    nc.tensor.transpose(psq[:], xcb[:, q, :], identb[:])
                    nc.vector.tensor_copy(xcT[:, q * P:(q + 1) * P], psq[:])

                # x (all rows, every core has f) -> xbT via 64 local transposes
                xb_nat = p0.tile([P, NS, D], BF16)
                for s in range(NS):
                    nc.vector.tensor_scalar(
                        xb_nat[:, s, :], f_sb[:, s, :], rn[:, s:s + 1], None,
                        ALU.mult,
                    )
                for s in range(NS):
                    psq = ps0.tile([P, P], BF16, tag="ptr", bufs=2)
                    nc.tensor.transpose(psq[:], xb_nat[:, s, :], identb[:])
                    nc.vector.tensor_copy(xbT[:, s * P:(s + 1) * P], psq[:])

                nc.scalar.activation(_r(fb32[:]), _r(f_sb[:]), AF.Copy, scale=1 / 32)
                nc.scalar.activation(
                    _r(fbc32[:]), _r(fc_sb[:]), AF.Copy, scale=1 / 32
                )

            # ================ phase 2+3: sim, topk threshold, M' ================
            with (
                tc.tile_pool(name="p23", bufs=1) as p23,
                tc.tile_pool(name="ps23", bufs=1, space="PSUM") as psp,
            ):
                for qt in range(NQ):
                    sim_sb = p23.tile([P, 16, 512], BF16, tag="sim", bufs=2)
                    for chp in range(8):
                        pssim = psp.tile([P, 2, 512], F32, tag="pssim", bufs=3)
                        for u in range(2):
                            ch = chp * 2 + u
                            nc.tensor.matmul(
                                pssim[:, u, :],
                                xcT[:, qt * P:(qt + 1) * P],
                                xbT[:, ch * 512:(ch + 1) * 512],
                                start=True, stop=True,
                            )
                        nc.scalar.activation(
                            sim_sb[:].rearrange("p c f -> p (c f)")
                            [:, chp * 1024:(chp + 1) * 1024],
                            pssim[:].rearrange("p c f -> p (c f)"),
                            AF.Copy,
                        )
                    simf = sim_sb[:].rearrange("p c f -> p (c f)")

                    cand = p23.tile([P, 8, 8], BF16, tag="cand", bufs=2)
                    for c in range(8):
                        nc.vector.max(
                            cand[:, c, :], simf[:, c * 1024:(c + 1) * 1024]
                        )
                    candf = cand[:].rearrange("p c f -> p (c f)")
                    m8 = None
                    for rnd in range(5):
                        m8 = p23.tile([P, 8], BF16, tag="m8", bufs=6)
                        nc.vector.max(m8[:], candf)
                        if rnd < 4:
                            nc.vector.match_replace(candf, m8[:], candf, -1e30)
                    th = p23.tile([P, 1], F32, tag="th", bufs=2)
                    nc.vector.tensor_tensor(th[:], m8[:, 0:1], m8[:, 1:2], ALU.add)
                    nc.vector.tensor_copy(tcol[:, qt:qt + 1], th[:])
                    Mt = p23.tile([P, NS, D], BF16, tag="Mt", bufs=2)
                    if qt % 2 == 0:
                        # ACT route: sigmoid step with bias = -1e9 * t''
                        tneg = p23.tile([P, 1], F32, tag="tneg", bufs=2)
                        nc.vector.tensor_scalar(
                            tneg[:], th[:], -0.5e9 * SHIFT, None, ALU.mult
                        )
                        nc.scalar.activation(
                            _r(Mt[:]), simf, AF.Sigmoid, bias=tneg[:], scale=1e9
                        )
                    else:
                        # DVE route: exact compare sim >= t''
                        tpos = p23.tile([P, 1], F32, tag="tneg", bufs=2)
                        nc.vector.tensor_scalar(
                            tpos[:], th[:], 0.5 * SHIFT, None, ALU.mult
                        )
                        nc.vector.tensor_scalar(
                            _r(Mt[:]), simf, tpos[:], None, ALU.is_ge
                        )
                    nc.scalar.dma_start(M_dram[qt * P:(qt + 1) * P, :], _r(Mt[:]))
                    del simf

            # ---- threshold row layout: negtb [1, BC], T_bc [P, BC] ----
            with (
                tc.tile_pool(name="throw", bufs=1) as trw,
                tc.tile_pool(name="pst", bufs=1, space="PSUM") as pst,
            ):
                tposb = trw.tile([1, BC], BF16)
                for q in range(NQ):
                    ps1 = pst.tile([1, P], F32, tag="t1", bufs=2)
                    nc.tensor.transpose(ps1[:], tcol[:, q:q + 1], identf[:])
                    nc.vector.tensor_scalar(
                        negtb[0:1, q * P:(q + 1) * P], ps1[:],
                        -0.5 * SHIFT, None, ALU.mult,
                    )
                    nc.vector.tensor_scalar(
                        tposb[0:1, q * P:(q + 1) * P], ps1[:],
                        0.5 * SHIFT, None, ALU.mult,
                    )
                for h in range(2):
                    psb = pst.tile([P, 512], F32, tag="tb", bufs=2)
                    nc.tensor.matmul(
                        psb[:], ones1b[:], tposb[0:1, h * 512:(h + 1) * 512],
                        start=True, stop=True,
                    )
                    nc.scalar.activation(
                        T_bc[:, h * 512:(h + 1) * 512], psb[:], AF.Copy
                    )

            # ================ diffusion (x2) ================
            def diffusion(i, dpool, rhsc_t, den_t, den_src, csrc, cscale,
                          gscale=1.0, do_ag=True):
                """one step: returns dc = (A @ src)_rows-of-core (fp32).

                rhsc_t [P,NQ,D] bf16: src_c/32 (scatter stationary)
                den_t  [P,NS,D] bf16: src/32 (or src; gather stationary)
                den_src: if not None, DRAM ap to load den_t from (after the
                         scatter is issued; sync queue so nothing blocks)
                csrc/cscale: merge-time correction, adds cscale*csrc
                """
                # ---- scatter: St[d, j] = sum_i (src_c/32)[i, d] * M'[i, j]
                S_sb = dpool.tile([P, NS, D], BF16, tag="Ssb")
                with tc.tile_pool(name=f"pscat{i}", bufs=1, space="PSUM") as psc:
                    for half in range(2):
                        psSt = [
                            psc.tile([P, 512], F32, tag="acc", bufs=8,
                                     name=f"psSt{i}_{half}_{js}")
                            for js in range(8)
                        ]
                        for q in range(NQ):
                            Mq = dpool.tile([P, 4096], BF16, tag="Mq", bufs=2)
                            nc.scalar.dma_start(
                                Mq[:],
                                M_dram[q * P:(q + 1) * P,
                                       half * 4096:(half + 1) * 4096],
                            )
                            for js in range(8):
                                nc.tensor.matmul(
                                    psSt[js][:], rhsc_t[:, q, :],
                                    Mq[:, js * 512:(js + 1) * 512],
                                    start=(q == 0), stop=(q == NQ - 1),
                                )
                        StT = dpool.tile([P, 8, 512], BF16, tag="StT", bufs=1)
                        for js in range(8):
                            if js % 2 == 0:
                                nc.scalar.activation(
                                    StT[:, js, :], psSt[js][:], AF.Copy
                                )
                            else:
                                nc.vector.tensor_copy(StT[:, js, :], psSt[js][:])
                        StTf = StT[:].rearrange("p a b -> p (a b)")
                        for b in range(32):
                            jg = half * 32 + b
                            psb = psc.tile([P, P], BF16, tag="acc", bufs=8,
                                           name=f"ptrS{i}_{jg}")
                            nc.tensor.transpose(
                                psb[:], StTf[:, b * P:(b + 1) * P], identb[:]
                            )
                            if b % 2 == 0:
                                nc.scalar.activation(S_sb[:, jg, :], psb[:], AF.Copy)
                            else:
                                nc.vector.tensor_copy(S_sb[:, jg, :], psb[:])
                nc.scalar.dma_start(_nat(s_bounce[i][:]), S_sb[:])
                nc.gpsimd.collective_compute(
                    "ReduceScatter", ALU.add, replica_groups=RG,
                    ins=[s_bounce[i][:].opt()], outs=[rs_out[i][:].opt()],
                )

                if den_src is not None:
                    nc.sync.dma_start(den_t[:], _nat(den_src[:]))

                # ---- gather: Gt[d, i] = sum_j src[j, d] * M'[i, j]
                # pass 0: M'^T j-slices recomputed from sim^T = xbT_js^T @ xcT
                # and cached in fp8 (exact for a 0/1 mask); pass 1 reuses them.
                with tc.tile_pool(name=f"pgat{i}", bufs=1, space="PSUM") as psg:
                    psGt = [
                        psg.tile([P, 512], F32, tag="gacc", bufs=2,
                                 name=f"psGt{i}_{h}")
                        for h in range(2)
                    ]
                    for js in range(NS):
                        MT = mtc[js]
                        if i == 0:
                            for h in range(2):
                                rg = psg.tile([P, 512], F32, tag="rg", bufs=4)
                                if js % 2 == 0:
                                    nc.tensor.matmul(
                                        rg[:], xbT[:, js * P:(js + 1) * P],
                                        xcT[:, h * 512:(h + 1) * 512],
                                        start=True, stop=False,
                                    )
                                    nc.tensor.matmul(
                                        rg[:], ones1b[:],
                                        negtb[0:1, h * 512:(h + 1) * 512],
                                        start=False, stop=True,
                                    )
                                    nc.scalar.activation(
                                        MT[:, h * 512:(h + 1) * 512], rg[:],
                                        AF.Sigmoid, scale=1e9,
                                    )
                                else:
                                    nc.tensor.matmul(
                                        rg[:], xbT[:, js * P:(js + 1) * P],
                                        xcT[:, h * 512:(h + 1) * 512],
                                        start=True, stop=True,
                                    )
                                    nc.vector.tensor_tensor(
                                        MT[:, h * 512:(h + 1) * 512], rg[:],
                                        T_bc[:, h * 512:(h + 1) * 512],
                                        ALU.is_ge,
                                    )
                        for h in range(2):
                            nc.tensor.matmul(
                                psGt[h][:], den_t[:, js, :],
                                MT[:, h * 512:(h + 1) * 512],
                                start=(js == 0), stop=(js == NS - 1),
                            )
                    GT = dpool.tile([P, 2, 512], F32, tag="GT", bufs=1)
                    for h in range(2):
                        nc.scalar.activation(GT[:, h, :], psGt[h][:], AF.Copy)
                    GTf = GT[:].rearrange("p a b -> p (a b)")

                    # rs + correction, then dc = G^T-transposed + that
                    rs_sb = dpool.tile([P, NQ, D], BF16, tag="rssb", bufs=2)
                    nc.sync.dma_start(rs_sb[:], _nat(rs_out[i][:]))
                    rsm = dpool.tile([P, NQ, D], F32, tag="rsm", bufs=2)
                    for q in range(NQ):
                        nc.vector.scalar_tensor_tensor(
                            rsm[:, q, :], csrc[:, q, :], cscale, rs_sb[:, q, :],
                            ALU.mult, ALU.add,
                        )
                    dc = dpool.tile([P, NQ, D], F32, tag=f"dc{i}")
                    for q in range(NQ):
                        psb = psg.tile([P, P], F32, tag="rg", bufs=4,
                                       name=f"ptrG{i}_{q}")
                        nc.tensor.transpose(
                            psb[:], GTf[:, q * P:(q + 1) * P], identf[:]
                        )
                        nc.vector.scalar_tensor_tensor(
                            dc[:, q, :], psb[:], gscale, rsm[:, q, :],
                            ALU.mult, ALU.add,
                        )
                if do_ag:
                    dcb = dpool.tile([P, NQ, D], BF16, tag="dcb", bufs=2)
                    nc.scalar.activation(_r(dcb[:]), _r(dc[:]), AF.Copy)
                    nc.scalar.dma_start(_nat(dcb_dram[:]), dcb[:])
                    nc.gpsimd.collective_compute(
                        "AllGather", ALU.bypass, replica_groups=RG,
                        ins=[dcb_dram[:].opt()], outs=[den_ag[:].opt()],
                    )
                return dc

            with tc.tile_pool(name="dif", bufs=1) as dpool:
                dc1 = diffusion(0, dpool, fbc32, fb32, None, fbc32, -2.0)
                # operands for pass 2 (diff1 arrives bf16 via AllGather);
                # den2 stays unscaled, the gather merge divides by 32.
                rhsc2 = dpool.tile([P, NQ, D], BF16, tag="rhsc2")
                nc.scalar.activation(_r(rhsc2[:]), _r(dc1[:]), AF.Copy, scale=1 / 32)
                den2 = dpool.tile([P, NS, D], BF16, tag="den2")

                dc2 = diffusion(1, dpool, rhsc2, den2, den_ag, dc1, -1.0 / 16.0,
                                gscale=1.0 / 32.0, do_ag=False)

                # ---- phase 7: z_c = x_c + 0.1 * geo_c/||geo_c||; AllGather z^T
                n2g = dpool.tile([P, NQ], F32)
                for q in range(NQ):
                    sq = dpool.tile([P, D], F32, tag="sqg", bufs=2)
                    nc.vector.scalar_tensor_tensor(
                        sq[:], dc2[:, q, :], 1.0, dc2[:, q, :],
                        ALU.mult, ALU.mult, accum_out=n2g[:, q:q + 1],
                    )
                ng = dpool.tile([P, NQ], F32)
                nc.scalar.activation(ng[:], n2g[:], AF.Sqrt)
                rgn = dpool.tile([P, NQ], F32)
                nc.vector.reciprocal(rgn[:], ng[:])
                rg01 = dpool.tile([P, NQ], F32)
                nc.vector.tensor_scalar(rg01[:], rgn[:], 0.1, None, ALU.mult)
                zbc = dpool.tile([P, NQ, D], BF16)
                for q in range(NQ):
                    nc.vector.scalar_tensor_tensor(
                        zbc[:, q, :], dc2[:, q, :], rg01[:, q:q + 1],
                        xc_nat[:, q, :], ALU.mult, ALU.add,
                    )
                zcT = dpool.tile([P, BC], BF16, tag="zcT")
                with tc.tile_pool(name="psz", bufs=1, space="PSUM") as psz:
                    for q in range(NQ):
                        psq = psz.tile([P, P], BF16, tag="ptr", bufs=2)
                        nc.tensor.transpose(psq[:], zbc[:, q, :], identb[:])
                        nc.vector.tensor_copy(zcT[:, q * P:(q + 1) * P], psq[:])
                nc.scalar.dma_start(zct_dram[:], zcT[:])
                nc.gpsimd.collective_compute(
                    "AllGather", ALU.bypass, replica_groups=RG,
                    ins=[zct_dram[:].opt()], outs=[zt_ag[:].opt()],
                )

            # ================ phase 8: softmax attention ================
            with (
                tc.tile_pool(name="p8", bufs=1) as p8,
                tc.tile_pool(name="ps8", bufs=1, space="PSUM") as psp,
            ):
                zT = p8.tile([P, B], BF16)
                for c in range(NCORES):
                    nc.sync.dma_start(
                        zT[:, c * BC:(c + 1) * BC],
                        zt_ag[c * P:(c + 1) * P, :],
                    )
                # keep the PE HAM window busy while the z AllGather lands so
                # the score matmuls start at 2.4 GHz, not 1.2
                wsc = p8.tile([P, P], BF16, tag="warm")
                for w in range(24):
                    psw = psp.tile([P, P], BF16, tag="psw", bufs=2)
                    nc.tensor.transpose(psw[:], identb[:], identb[:])
                    if w == 23:
                        nc.vector.tensor_copy(wsc[:], psw[:])
                for grp in range(2):
                    psOUT = psp.tile([P, 512], F32, tag="psOUT", bufs=1)
                    sacc = p8.tile([P, 512], F32, tag="sacc", bufs=2)
                    nc.vector.memset(sacc[:], 0.0)
                    for jc in range(NS):
                        psT = psp.tile([P, 512], F32, tag="psT", bufs=3)
                        nc.tensor.matmul(
                            psT[:], zT[:, jc * P:(jc + 1) * P],
                            xcT[:, grp * 512:(grp + 1) * 512],
                            start=True, stop=True,
                        )
                        Pt = p8.tile([P, 512], BF16, tag="Pt", bufs=4)
                        nc.scalar.activation(Pt[:], psT[:], AF.Exp, scale=10.0)
                        nc.tensor.matmul(
                            psOUT[:], fb32[:, jc, :], Pt[:],
                            start=(jc == 0), stop=(jc == NS - 1),
                        )
                        nc.vector.scalar_tensor_tensor(
                            sacc[:], Pt[:], 1.0, sacc[:], ALU.mult, ALU.add,
                        )
                    # denom: reduce sacc over partitions via 4 PE transposes
                    s1nat = p8.tile([P, 4], F32, tag="s1nat", bufs=2)
                    scr = p8.tile([P, P], F32, tag="scr", bufs=2)
                    for b in range(4):
                        psB = psp.tile([P, P], F32, tag="psB", bufs=2)
                        nc.tensor.transpose(
                            psB[:], sacc[:, b * P:(b + 1) * P], identf[:]
                        )
                        nc.vector.tensor_scalar(
                            scr[:], psB[:], 1.0, 0.0, ALU.mult, ALU.add,
                            accum_out=s1nat[:, b:b + 1],
                        )
                    rnat = p8.tile([P, 4], F32, tag="rnat", bufs=2)
                    nc.vector.reciprocal(rnat[:], s1nat[:])
                    rnat32 = p8.tile([P, 4], F32, tag="rnat32", bufs=2)
                    nc.vector.tensor_scalar(
                        rnat32[:], rnat[:], 32.0, None, ALU.mult
                    )
                    OUT_sb = p8.tile([P, 512], F32, tag="OUTsb", bufs=2)
                    nc.scalar.activation(OUT_sb[:], psOUT[:], AF.Copy)
                    for b in range(4):
                        psB = psp.tile([P, P], F32, tag="psB", bufs=2)
                        nc.tensor.transpose(
                            psB[:], OUT_sb[:, b * P:(b + 1) * P], identf[:]
                        )
                        ob = p8.tile([P, D], F32, tag="ob", bufs=2)
                        nc.vector.tensor_scalar(
                            ob[:], psB[:], rnat32[:, b:b + 1], None, ALU.mult
                        )
                        nc.scalar.dma_start(
                            out_ext[grp * 512 + b * P: grp * 512 + (b + 1) * P, :],
                            ob[:],
                        )

    nc.finalize()
    return nc


_NC_CACHE = None


def kernel(features: np.ndarray) -> np.ndarray:
    global _NC_CACHE
    features = np.ascontiguousarray(np.asarray(features, np.float32))
    assert features.shape == (B, D), features.shape
    if _NC_CACHE is None:
        _NC_CACHE = build()
    in_maps = [
        {
            "feat": features,
            "featc": features[c * BC:(c + 1) * BC].copy(),
        }
        for c in range(NCORES)
    ]
    res = run_bass_kernel_spmd(_NC_CACHE, in_maps, core_ids=list(range(NCORES)))
    return np.concatenate(
        [np.asarray(res.results[c]["out"], np.float32) for c in range(NCORES)],
        axis=0,
    )


# revision 14
# speedup vs baseline: 1.3098x; 1.0479x over previous
"""Trainium2 Bass kernel for nn_ALMSLayer (gnn_message_passing), 8 NeuronCores.

Algorithm (per core c, rows R_c = [c*1024, (c+1)*1024) of B=8192):
  x       = f / ||f||                      (rows normalized)
  sim     = x_c @ x^T                      (bf16 matmul, [1024, 8192])
  topk    : per row, 33rd/34th-largest via chunked top-8 candidates ->
            threshold t; M' = (sim >= t) in {0,1} (includes self edge)
  A       = (M' + M'^T)/32 - I/16          (-I/16 corrects the self edges)
  diff1   = A @ f ; geodesic = A @ diff1   (dense bf16 matmuls; M'^T side
            summed across cores with ReduceScatter, M' side local, full
            tensors rebuilt with a bf16 AllGather)
  z       = x + 0.1 * geodesic/||geodesic||
  out     = softmax((x_c @ z^T)/0.1) @ f   (flash-attention style)

Key perf structure vs the naive layout:
  * M' is written to DRAM once (natural layout) and read back only for the
    scatter side.  The gather side needs M'^T; instead of transposed DMA
    reads (2-byte-granularity crawl), each M'^T j-slice is recomputed on
    the fly: sim^T[j, i] = x_j . x_i via one PE matmul from xbT/xcT, with
    the per-i threshold applied either by a rank-1 PSUM accumulate plus
    sigmoid step (even slices) or a broadcast-threshold compare (odd).
  * x^T and z^T full tensors are built by AllGathering locally PE-transposed
    [D, 1024] chunks (natural-layout loads), never by DMA transpose.
  * diff2's scatter is issued before diff1's AllGather completes; loads that
    depend on collectives go on the sync queue so the scalar queue never
    blocks.

Host side only shards/replicates/concats; all arithmetic on device.
"""
import sys

sys.path.insert(0, "/opt/trn_rl_repo")

import numpy as np

import concourse.bass as bass
import concourse.tile as tile
from concourse import bacc, mybir
from concourse.bass_utils import run_bass_kernel_spmd

F32 = mybir.dt.float32
BF16 = mybir.dt.bfloat16
FP8 = mybir.dt.float8e4
AF = mybir.ActivationFunctionType
ALU = mybir.AluOpType

B = 8192          # nodes
D = 128           # feature dim
P = 128           # partitions
NCORES = 8
BC = B // NCORES  # rows per core (1024)
NS = B // P       # 64 j-slices of 128 rows
NQ = BC // P      # 8 q-tiles per core
RG = [list(range(NCORES))]

# threshold shift: t'' = t*(1 - 2^-10) so elements equal to the 33rd value
# land strictly above the threshold (bf16 value gaps are either 0 or
# >= ~2^-9 relative, so the shift never misclassifies rank 34).
SHIFT = 1.0 - 2.0 ** -10


def _r(ap):
    return ap.rearrange("p s d -> p (s d)")


def _nat(dram_ap):
    """DRAM [S*P, D] viewed as SBUF-natural [P, S, D] (row j = s*128+p)."""
    return dram_ap.rearrange("(s p) d -> p s d", p=P)


def build():
    nc = bacc.Bacc(None, target_bir_lowering=False, debug=False)

    feat = nc.declare_dram_parameter("feat", [B, D], F32, isOutput=False)
    featc = nc.declare_dram_parameter("featc", [BC, D], F32, isOutput=False)
    out_ext = nc.declare_dram_parameter("out", [BC, D], F32, isOutput=True)

    with tile.TileContext(nc) as tc:
        with (
            tc.tile_pool(name="dram", bufs=1, space="DRAM") as dr,
            tc.tile_pool(name="pers", bufs=1) as pers,
        ):
            # ---------------- DRAM scratch ----------------
            M_dram = dr.tile([BC, B], BF16)
            s_bounce = [dr.tile([B, D], BF16, name=f"sbounce{i}") for i in range(2)]
            rs_out = [dr.tile([BC, D], BF16, name=f"rsout{i}") for i in range(2)]
            zct_dram = dr.tile([P, BC], BF16)
            dcb_dram = dr.tile([BC, D], BF16)
            zt_ag = dr.tile([NCORES * P, BC], BF16, addr_space="Shared")
            den_ag = dr.tile([B, D], BF16, addr_space="Shared")
            skew_in = dr.tile([8, 16], BF16)
            skew_out = dr.tile([64, 16], BF16, addr_space="Shared")

            # ---------------- persistent SBUF ----------------
            identf = pers.tile([P, P], F32)
            identb = pers.tile([P, P], BF16)
            ones_col = pers.tile([P, 1], BF16)
            ones1b = pers.tile([1, P], BF16)
            fb32 = pers.tile([P, NS, D], BF16)       # f/32 (gather rhs + phase-8 V)
            fbc32 = pers.tile([P, NQ, D], BF16)      # f_c/32 (scatter-1 stationary)
            xc_nat = pers.tile([P, NQ, D], F32)      # x_c fp32 (z build)
            xcT = pers.tile([P, BC], BF16)           # x_c^T [d, i]
            xbT = pers.tile([P, B], BF16)            # x^T [d, j] (AllGathered)
            tcol = pers.tile([P, NQ], F32)           # raw th (v33+v34) per q-tile
            negtb = pers.tile([1, BC], BF16)         # -t'' per i (rank-1 rhs)
            T_bc = pers.tile([P, BC], BF16)          # +t'' broadcast to all parts
            # transposed-mask cache: M'^T j-slices in fp8 (0/1 exact), built
            # during diffusion-1's gather, reused verbatim by diffusion-2
            mtc = [
                pers.tile([P, BC], FP8, name=f"mtc{js}") for js in range(NS)
            ]

            nc.vector.memset(identf[:], 1.0)
            nc.gpsimd.affine_select(
                identf[:], identf[:], pattern=[[1, P]], compare_op=ALU.is_equal,
                fill=0.0, base=0, channel_multiplier=-1,
            )
            nc.vector.memset(identb[:], 1.0)
            nc.gpsimd.affine_select(
                identb[:], identb[:], pattern=[[1, P]], compare_op=ALU.is_equal,
                fill=0.0, base=0, channel_multiplier=-1,
            )
            nc.vector.memset(ones_col[:], 1.0)
            nc.vector.memset(ones1b[:], 1.0)

            # ================ phase 0: load, normalize, layouts ================
            with (
                tc.tile_pool(name="p0", bufs=1) as p0,
                tc.tile_pool(name="ps0", bufs=1, space="PSUM") as ps0,
            ):
                fc_sb = p0.tile([P, NQ, D], F32)
                nc.scalar.dma_start(fc_sb[:], _nat(featc[:]))
                f_sb = p0.tile([P, NS, D], F32)
                nc.scalar.dma_start(f_sb[:], _nat(feat[:]))

                # dummy collective: absorbs inter-core launch skew here (its
                # output is never consumed) so the first real collective
                # doesn't pay it
                skw = p0.tile([8, 16], BF16, tag="skw")
                nc.vector.memset(skw[:], 0.0)
                nc.sync.dma_start(skew_in[:], skw[:])
                nc.gpsimd.collective_compute(
                    "AllGather", ALU.bypass, replica_groups=RG,
                    ins=[skew_in[:].opt()], outs=[skew_out[:].opt()],
                )

                # row norms via DVE squares with accumulate
                n2c = p0.tile([P, NQ], F32)
                for q in range(NQ):
                    sq = p0.tile([P, D], F32, tag="sq", bufs=2)
                    nc.vector.scalar_tensor_tensor(
                        sq[:], fc_sb[:, q, :], 1.0, fc_sb[:, q, :],
                        ALU.mult, ALU.mult, accum_out=n2c[:, q:q + 1],
                    )
                nrmc = p0.tile([P, NQ], F32)
                nc.scalar.activation(nrmc[:], n2c[:], AF.Sqrt)
                rnc = p0.tile([P, NQ], F32)
                nc.vector.reciprocal(rnc[:], nrmc[:])
                n2 = p0.tile([P, NS], F32)
                for s in range(NS):
                    sq = p0.tile([P, D], F32, tag="sq", bufs=2)
                    nc.vector.scalar_tensor_tensor(
                        sq[:], f_sb[:, s, :], 1.0, f_sb[:, s, :],
                        ALU.mult, ALU.mult, accum_out=n2[:, s:s + 1],
                    )
                nrm = p0.tile([P, NS], F32)
                nc.scalar.activation(nrm[:], n2[:], AF.Sqrt)
                rn = p0.tile([P, NS], F32)
                nc.vector.reciprocal(rn[:], nrm[:])

                # x_c fp32 + bf16; xcT via PE transposes
                for q in range(NQ):
                    nc.vector.tensor_scalar(
                        xc_nat[:, q, :], fc_sb[:, q, :], rnc[:, q:q + 1], None,
                        ALU.mult,
                    )
                xcb = p0.tile([P, NQ, D], BF16)
                nc.scalar.activation(_r(xcb[:]), _r(xc_nat[:]), AF.Copy)
                for q in range(NQ):
                    psq = ps0.tile([P, P], BF16, tag="ptr", bufs=2)
                    nc.tensor.transpose(psq[:], xcb[:, q, :], identb[:])
                    nc.vector.tensor_copy(xcT[:, q * P:(q + 1) * P], psq[:])

                # x (all rows, every core has f) -> xbT via 64 local transposes
                xb_nat = p0.tile([P, NS, D], BF16)
                for s in range(NS):
                    nc.vector.tensor_scalar(
                        xb_nat[:, s, :], f_sb[:, s, :], rn[:, s:s + 1], None,
                        ALU.mult,
                    )
                for s in range(NS):
                    psq = ps0.tile([P, P], BF16, tag="ptr", bufs=2)
                    nc.tensor.transpose(psq[:], xb_nat[:, s, :], identb[:])
                    nc.vector.tensor_copy(xbT[:, s * P:(s + 1) * P], psq[:])

                nc.scalar.activation(_r(fb32[:]), _r(f_sb[:]), AF.Copy, scale=1 / 32)
                nc.scalar.activation(
                    _r(fbc32[:]), _r(fc_sb[:]), AF.Copy, scale=1 / 32
                )

            # ================ phase 2+3: sim, topk threshold, M' ================
            with (
                tc.tile_pool(name="p23", bufs=1) as p23,
                tc.tile_pool(name="ps23", bufs=1, space="PSUM") as psp,
            ):
                for qt in range(NQ):
                    sim_sb = p23.tile([P, 16, 512], BF16, tag="sim", bufs=2)
                    for chp in range(8):
                        pssim = psp.tile([P, 2, 512], F32, tag="pssim", bufs=3)
                        for u in range(2):
                            ch = chp * 2 + u
                            nc.tensor.matmul(
                                pssim[:, u, :],
                                xcT[:, qt * P:(qt + 1) * P],
                                xbT[:, ch * 512:(ch + 1) * 512],
                                start=True, stop=True,
                            )
                        nc.scalar.activation(
                            sim_sb[:].rearrange("p c f -> p (c f)")
                            [:, chp * 1024:(chp + 1) * 1024],
                            pssim[:].rearrange("p c f -> p (c f)"),
                            AF.Copy,
                        )
                    simf = sim_sb[:].rearrange("p c f -> p (c f)")

                    cand = p23.tile([P, 8, 8], BF16, tag="cand", bufs=2)
                    for c in range(8):
                        nc.vector.max(
                            cand[:, c, :], simf[:, c * 1024:(c + 1) * 1024]
                        )
                    candf = cand[:].rearrange("p c f -> p (c f)")
                    m8 = None
                    for rnd in range(5):
                        m8 = p23.tile([P, 8], BF16, tag="m8", bufs=6)
                        nc.vector.max(m8[:], candf)
                        if rnd < 4:
                            nc.vector.match_replace(candf, m8[:], candf, -1e30)
                    th = p23.tile([P, 1], F32, tag="th", bufs=2)
                    nc.vector.tensor_tensor(th[:], m8[:, 0:1], m8[:, 1:2], ALU.add)
                    nc.vector.tensor_copy(tcol[:, qt:qt + 1], th[:])
                    Mt = p23.tile([P, NS, D], BF16, tag="Mt", bufs=2)
                    if qt % 2 == 0:
                        # ACT route: sigmoid step with bias = -1e9 * t''
                        tneg = p23.tile([P, 1], F32, tag="tneg", bufs=2)
                        nc.vector.tensor_scalar(
                            tneg[:], th[:], -0.5e9 * SHIFT, None, ALU.mult
                        )
                        nc.scalar.activation(
                            _r(Mt[:]), simf, AF.Sigmoid, bias=tneg[:], scale=1e9
                        )
                    else:
                        # DVE route: exact compare sim >= t''
                        tpos = p23.tile([P, 1], F32, tag="tneg", bufs=2)
                        nc.vector.tensor_scalar(
                            tpos[:], th[:], 0.5 * SHIFT, None, ALU.mult
                        )
                        nc.vector.tensor_scalar(
                            _r(Mt[:]), simf, tpos[:], None, ALU.is_ge
                        )
                    nc.scalar.dma_start(M_dram[qt * P:(qt + 1) * P, :], _r(Mt[:]))
                    del simf

            # ---- threshold row layout: negtb [1, BC], T_bc [P, BC] ----
            with (
                tc.tile_pool(name="throw", bufs=1) as trw,
                tc.tile_pool(name="pst", bufs=1, space="PSUM") as pst,
            ):
                tposb = trw.tile([1, BC], BF16)
                for q in range(NQ):
                    ps1 = pst.tile([1, P], F32, tag="t1", bufs=2)
                    nc.tensor.transpose(ps1[:], tcol[:, q:q + 1], identf[:])
                    nc.vector.tensor_scalar(
                        negtb[0:1, q * P:(q + 1) * P], ps1[:],
                        -0.5 * SHIFT, None, ALU.mult,
                    )
                    nc.vector.tensor_scalar(
                        tposb[0:1, q * P:(q + 1) * P], ps1[:],
                        0.5 * SHIFT, None, ALU.mult,
                    )
                for h in range(2):
                    psb = pst.tile([P, 512], F32, tag="tb", bufs=2)
                    nc.tensor.matmul(
                        psb[:], ones1b[:], tposb[0:1, h * 512:(h + 1) * 512],
                        start=True, stop=True,
                    )
                    nc.scalar.activation(
                        T_bc[:, h * 512:(h + 1) * 512], psb[:], AF.Copy
                    )

            # ================ diffusion (x2) ================
            def diffusion(i, dpool, rhsc_t, den_t, den_src, csrc, cscale,
                          gscale=1.0, do_ag=True):
                """one step: returns dc = (A @ src)_rows-of-core (fp32).

                rhsc_t [P,NQ,D] bf16: src_c/32 (scatter stationary)
                den_t  [P,NS,D] bf16: src/32 (or src; gather stationary)
                den_src: if not None, DRAM ap to load den_t from (after the
                         scatter is issued; sync queue so nothing blocks)
                csrc/cscale: merge-time correction, adds cscale*csrc
                """
                # ---- scatter: St[d, j] = sum_i (src_c/32)[i, d] * M'[i, j]
                S_sb = dpool.tile([P, NS, D], BF16, tag="Ssb")
                with tc.tile_pool(name=f"pscat{i}", bufs=1, space="PSUM") as psc:
                    for half in range(2):
                        psSt = [
                            psc.tile([P, 512], F32, tag="acc", bufs=8,
                                     name=f"psSt{i}_{half}_{js}")
                            for js in range(8)
                        ]
                        for q in range(NQ):
                            Mq = dpool.tile([P, 4096], BF16, tag="Mq", bufs=2)
                            nc.scalar.dma_start(
                                Mq[:],
                                M_dram[q * P:(q + 1) * P,
                                       half * 4096:(half + 1) * 4096],
                            )
                            for js in range(8):
                                nc.tensor.matmul(
                                    psSt[js][:], rhsc_t[:, q, :],
                                    Mq[:, js * 512:(js + 1) * 512],
                                    start=(q == 0), stop=(q == NQ - 1),
                                )
                        StT = dpool.tile([P, 8, 512], BF16, tag="StT", bufs=1)
                        for js in range(8):
                            if js % 2 == 0:
                                nc.scalar.activation(
                                    StT[:, js, :], psSt[js][:], AF.Copy
                                )
                            else:
                                nc.vector.tensor_copy(StT[:, js, :], psSt[js][:])
                        StTf = StT[:].rearrange("p a b -> p (a b)")
                        for b in range(32):
                            jg = half * 32 + b
                            psb = psc.tile([P, P], BF16, tag="acc", bufs=8,
                                           name=f"ptrS{i}_{jg}")
                            nc.tensor.transpose(
                                psb[:], StTf[:, b * P:(b + 1) * P], identb[:]
                            )
                            if b % 2 == 0:
                                nc.scalar.activation(S_sb[:, jg, :], psb[:], AF.Copy)
                            else:
                                nc.vector.tensor_copy(S_sb[:, jg, :], psb[:])
                nc.scalar.dma_start(_nat(s_bounce[i][:]), S_sb[:])
                nc.gpsimd.collective_compute(
                    "ReduceScatter", ALU.add, replica_groups=RG,
                    ins=[s_bounce[i][:].opt()], outs=[rs_out[i][:].opt()],
                )

                if den_src is not None:
                    nc.sync.dma_start(den_t[:], _nat(den_src[:]))

                # ---- gather: Gt[d, i] = sum_j src[j, d] * M'[i, j]
                # pass 0: M'^T j-slices recomputed from sim^T = xbT_js^T @ xcT
                # and cached in fp8 (exact for a 0/1 mask); pass 1 reuses them.
                with tc.tile_pool(name=f"pgat{i}", bufs=1, space="PSUM") as psg:
                    psGt = [
                        psg.tile([P, 512], F32, tag="gacc", bufs=2,
                                 name=f"psGt{i}_{h}")
                        for h in range(2)
                    ]
                    for js in range(NS):
                        MT = mtc[js]
                        if i == 0:
                            for h in range(2):
                                rg = psg.tile([P, 512], F32, tag="rg", bufs=4)
                                if js % 2 == 0:
                                    nc.tensor.matmul(
                                        rg[:], xbT[:, js * P:(js + 1) * P],
                                        xcT[:, h * 512:(h + 1) * 512],
                                        start=True, stop=False,
                                    )
                                    nc.tensor.matmul(
                                        rg[:], ones1b[:],
                                        negtb[0:1, h * 512:(h + 1) * 512],
                                        start=False, stop=True,
                                    )
                                    nc.scalar.activation(
                                        MT[:, h * 512:(h + 1) * 512], rg[:],
                                        AF.Sigmoid, scale=1e9,
                                    )
                                else:
                                    nc.tensor.matmul(
                                        rg[:], xbT[:, js * P:(js + 1) * P],
                                        xcT[:, h * 512:(h + 1) * 512],
                                        start=True, stop=True,
                                    )
                                    nc.vector.tensor_tensor(
                                        MT[:, h * 512:(h + 1) * 512], rg[:],
                                        T_bc[:, h * 512:(h + 1) * 512],
                                        ALU.is_ge,
                                    )
                        for h in range(2):
                            nc.tensor.matmul(
                                psGt[h][:], den_t[:, js, :],
                                MT[:, h * 512:(h + 1) * 512],
                                start=(js == 0), stop=(js == NS - 1),
                            )
                    # keep the merge (which waits on the ReduceScatter) from
                    # being scheduled into the js loop's queues — a slow
                    # collective at a queue head stalls every engine
                    tc.no_sync_barrier()
                    GT = dpool.tile([P, 2, 512], F32, tag="GT", bufs=1)
                    for h in range(2):
                        nc.scalar.activation(GT[:, h, :], psGt[h][:], AF.Copy)
                    GTf = GT[:].rearrange("p a b -> p (a b)")

                    # rs + correction, then dc = G^T-transposed + that
                    rs_sb = dpool.tile([P, NQ, D], BF16, tag="rssb", bufs=2)
                    nc.sync.dma_start(rs_sb[:], _nat(rs_out[i][:]))
                    rsm = dpool.tile([P, NQ, D], F32, tag="rsm", bufs=2)
                    for q in range(NQ):
                        nc.vector.scalar_tensor_tensor(
                            rsm[:, q, :], csrc[:, q, :], cscale, rs_sb[:, q, :],
                            ALU.mult, ALU.add,
                        )
                    dc = dpool.tile([P, NQ, D], F32, tag=f"dc{i}")
                    for q in range(NQ):
                        psb = psg.tile([P, P], F32, tag="rg", bufs=4,
                                       name=f"ptrG{i}_{q}")
                        nc.tensor.transpose(
                            psb[:], GTf[:, q * P:(q + 1) * P], identf[:]
                        )
                        nc.vector.scalar_tensor_tensor(
                            dc[:, q, :], psb[:], gscale, rsm[:, q, :],
                            ALU.mult, ALU.add,
                        )
                if do_ag:
                    dcb = dpool.tile([P, NQ, D], BF16, tag="dcb", bufs=2)
                    nc.scalar.activation(_r(dcb[:]), _r(dc[:]), AF.Copy)
                    nc.scalar.dma_start(_nat(dcb_dram[:]), dcb[:])
                    nc.gpsimd.collective_compute(
                        "AllGather", ALU.bypass, replica_groups=RG,
                        ins=[dcb_dram[:].opt()], outs=[den_ag[:].opt()],
                    )
                return dc

            with tc.tile_pool(name="dif", bufs=1) as dpool:
                dc1 = diffusion(0, dpool, fbc32, fb32, None, fbc32, -2.0)
                # operands for pass 2 (diff1 arrives bf16 via AllGather);
                # den2 stays unscaled, the gather merge divides by 32.
                rhsc2 = dpool.tile([P, NQ, D], BF16, tag="rhsc2")
                nc.scalar.activation(_r(rhsc2[:]), _r(dc1[:]), AF.Copy, scale=1 / 32)
                den2 = dpool.tile([P, NS, D], BF16, tag="den2")

                dc2 = diffusion(1, dpool, rhsc2, den2, den_ag, dc1, -1.0 / 16.0,
                                gscale=1.0 / 32.0, do_ag=False)

                # ---- phase 7: z_c = x_c + 0.1 * geo_c/||geo_c||; AllGather z^T
                n2g = dpool.tile([P, NQ], F32)
                for q in range(NQ):
                    sq = dpool.tile([P, D], F32, tag="sqg", bufs=2)
                    nc.vector.scalar_tensor_tensor(
                        sq[:], dc2[:, q, :], 1.0, dc2[:, q, :],
                        ALU.mult, ALU.mult, accum_out=n2g[:, q:q + 1],
                    )
                ng = dpool.tile([P, NQ], F32)
                nc.scalar.activation(ng[:], n2g[:], AF.Sqrt)
                rgn = dpool.tile([P, NQ], F32)
                nc.vector.reciprocal(rgn[:], ng[:])
                rg01 = dpool.tile([P, NQ], F32)
                nc.vector.tensor_scalar(rg01[:], rgn[:], 0.1, None, ALU.mult)
                zbc = dpool.tile([P, NQ, D], BF16)
                for q in range(NQ):
                    nc.vector.scalar_tensor_tensor(
                        zbc[:, q, :], dc2[:, q, :], rg01[:, q:q + 1],
                        xc_nat[:, q, :], ALU.mult, ALU.add,
                    )
                zcT = dpool.tile([P, BC], BF16, tag="zcT")
                with tc.tile_pool(name="psz", bufs=1, space="PSUM") as psz:
                    for q in range(NQ):
                        psq = psz.tile([P, P], BF16, tag="ptr", bufs=2)
                        nc.tensor.transpose(psq[:], zbc[:, q, :], identb[:])
                        nc.vector.tensor_copy(zcT[:, q * P:(q + 1) * P], psq[:])
                nc.scalar.dma_start(zct_dram[:], zcT[:])
                nc.gpsimd.collective_compute(
                    "AllGather", ALU.bypass, replica_groups=RG,
                    ins=[zct_dram[:].opt()], outs=[zt_ag[:].opt()],
                )

            # ================ phase 8: softmax attention ================
            with (
                tc.tile_pool(name="p8", bufs=1) as p8,
                tc.tile_pool(name="ps8", bufs=1, space="PSUM") as psp,
            ):
                zT = p8.tile([P, B], BF16)
                for c in range(NCORES):
                    nc.sync.dma_start(
                        zT[:, c * BC:(c + 1) * BC],
                        zt_ag[c * P:(c + 1) * P, :],
                    )
                # keep the PE HAM window busy while the z AllGather lands so
                # the score matmuls start at 2.4 GHz, not 1.2
                wsc = p8.tile([P, P], BF16, tag="warm")
                for w in range(24):
                    psw = psp.tile([P, P], BF16, tag="psw", bufs=2)
                    nc.tensor.transpose(psw[:], identb[:], identb[:])
                    if w == 23:
                        nc.vector.tensor_copy(wsc[:], psw[:])
                for grp in range(2):
                    psOUT = psp.tile([P, 512], F32, tag="psOUT", bufs=1)
                    sacc = p8.tile([P, 512], F32, tag="sacc", bufs=2)
                    nc.vector.memset(sacc[:], 0.0)
                    for jc in range(NS):
                        psT = psp.tile([P, 512], F32, tag="psT", bufs=3)
                        nc.tensor.matmul(
                            psT[:], zT[:, jc * P:(jc + 1) * P],
                            xcT[:, grp * 512:(grp + 1) * 512],
                            start=True, stop=True,
                        )
                        Pt = p8.tile([P, 512], BF16, tag="Pt", bufs=4)
                        nc.scalar.activation(Pt[:], psT[:], AF.Exp, scale=10.0)
                        nc.tensor.matmul(
                            psOUT[:], fb32[:, jc, :], Pt[:],
                            start=(jc == 0), stop=(jc == NS - 1),
                        )
                        nc.vector.scalar_tensor_tensor(
                            sacc[:], Pt[:], 1.0, sacc[:], ALU.mult, ALU.add,
                        )
                    # denom: reduce sacc over partitions via 4 PE transposes
                    s1nat = p8.tile([P, 4], F32, tag="s1nat", bufs=2)
                    scr = p8.tile([P, P], F32, tag="scr", bufs=2)
                    for b in range(4):
                        psB = psp.tile([P, P], F32, tag="psB", bufs=2)
                        nc.tensor.transpose(
                            psB[:], sacc[:, b * P:(b + 1) * P], identf[:]
                        )
                        nc.vector.tensor_scalar(
                            scr[:], psB[:], 1.0, 0.0, ALU.mult, ALU.add,
                            accum_out=s1nat[:, b:b + 1],
                        )
                    rnat = p8.tile([P, 4], F32, tag="rnat", bufs=2)
                    nc.vector.reciprocal(rnat[:], s1nat[:])
                    rnat32 = p8.tile([P, 4], F32, tag="rnat32", bufs=2)
                    nc.vector.tensor_scalar(
                        rnat32[:], rnat[:], 32.0, None, ALU.mult
                    )
                    OUT_sb = p8.tile([P, 512], F32, tag="OUTsb", bufs=2)
                    nc.scalar.activation(OUT_sb[:], psOUT[:], AF.Copy)
                    for b in range(4):
                        psB = psp.tile([P, P], F32, tag="psB", bufs=2)
                        nc.tensor.transpose(
                            psB[:], OUT_sb[:, b * P:(b + 1) * P], identf[:]
                        )
                        ob = p8.tile([P, D], F32, tag="ob", bufs=2)
                        nc.vector.tensor_scalar(
                            ob[:], psB[:], rnat32[:, b:b + 1], None, ALU.mult
                        )
                        nc.scalar.dma_start(
                            out_ext[grp * 512 + b * P: grp * 512 + (b + 1) * P, :],
                            ob[:],
                        )

    nc.finalize()
    return nc


_NC_CACHE = None


def kernel(features: np.ndarray) -> np.ndarray:
    global _NC_CACHE
    features = np.ascontiguousarray(np.asarray(features, np.float32))
    assert features.shape == (B, D), features.shape
    if _NC_CACHE is None:
        _NC_CACHE = build()
    in_maps = [
        {
            "feat": features,
            "featc": features[c * BC:(c + 1) * BC].copy(),
        }
        for c in range(NCORES)
    ]
    res = run_bass_kernel_spmd(_NC_CACHE, in_maps, core_ids=list(range(NCORES)))
    return np.concatenate(
        [np.asarray(res.results[c]["out"], np.float32) for c in range(NCORES)],
        axis=0,
    )
